# revision 27
# baseline (speedup 1.0000x reference)
"""MetaPathGNN Trainium kernel v8.

v7 (927us measured): layer A aggregates a host-pregathered x message stream;
one 18/31-block AllGather pair publishes layer-A output tables; layer B
dma_gathers per-edge rows (Q7 SWDGE desc-gen is the serial bottleneck:
~1.08us/call + 7.38ns/row, ~70k rows) and aggregates via one-hot matmuls.
Trace: gathers couldn't start until AG1 completed at ~241us; stream then ran
~670us nearly back-to-back.

v8 changes (target ~690us):
- The layer-A output table is split into 4 source-range tables
  (TB=[6,10,16,17] blocks). Each AllGather fires as soon as layer A finishes
  its block range, so the first gathers start at ~75us instead of ~241us.
  AG dispatches are interleaved into the gather call stream at tuned
  positions (AGPOS) because the in-order gpsimd queue would otherwise stall.
- Gather calls pack ~1000 rows (consecutive dest blocks) per call to
  amortize the ~1.08us fixed cost; per-(dest-block, table) row counts are
  padded only to the cross-core max (SPMD uniformity), not to tile
  boundaries: the call tail is -1 indices, which SWDGE skips at desc-gen
  time (num_idxs_reg = shared true count).
- Per-block aggregates accumulate across the 4 table phases in an SBUF f32
  accumulator (PSUM can't stay open across phases); z/LN/out for block b
  runs right after its table-3 call, so the tail work spreads over the
  final phase.
- Gather pool buffers are memset once at start (gpsimd, idle pre-AG0):
  rows past num_idxs_reg are never written by the DMA, and NaN garbage
  would poison the sentinel-masked one-hot matmuls (0*NaN=NaN).

Known hardware walls (measured v7):
- dma_gather: ~1.08us fixed + 7.38ns/row serial Q7 time, hard ~1024-row cap
  per call (SWDGE carveout; bigger scratch does NOT raise it, 2 queues do
  NOT parallelize - desc-gen serializes on the Pool engine).
- Interleaved PSUM accumulation groups in one bank CORRUPT results; keep
  accumulation groups sequential.
- Per-partition-contiguous DRAM layout for the stream ([B*P, T_A*H]) gives
  6KB descriptors (~line rate); naive layout caps at ~190GB/s.
- AllGather (RDH): ~90-200GB/s alg-bw + ~2-11us trigger latency; the CC
  dispatch instruction sem-waits in the gpsimd queue, and a gather call
  data-dependent on an un-flown AG blocks the whole in-order queue.
"""

import numpy as np
from contextlib import ExitStack

import concourse.bass as bass
import concourse.tile as tile
from concourse import bacc, mybir, library_config
from concourse.bass_utils import run_bass_kernel_spmd
from concourse.masks import make_identity

P = 128
F32 = mybir.dt.float32
F16 = mybir.dt.float16
I16 = mybir.dt.int16
NPF16 = np.float16
EPS = 1e-5

# ---- v8 tuning knobs
TB = (6, 10, 16, 17)      # blocks per layer-A output table (sum must be B=49)
ROWCAP = 1024             # rows per gather call (hard SWDGE carveout cap)
# AG k+1 dispatched after this many gather calls of table k have issued
# (must be late enough that layer A produced table k+1's blocks, early
# enough that AG k+1 completes before table k's gather stream drains).
AGOFF = (1, 5, 1)
MIX_T2 = 11  # table-2 calls deferred into table-3's phase (spreads finish-block PE)
GPOOL_BUFS = 18           # gather pool depth (calls in flight)
IDX_AHEAD = 4             # gather idx-tile JIT prefetch depth


def cdiv(a, b):
    return (a + b - 1) // b


# ---------------------------------------------------------------- host prep

def sort_edges_by_dest(e0, e1, ncores, npc):
    """Per core: edge (local_dest, src) arrays sorted by local dest."""
    e0 = np.asarray(e0).astype(np.int64)
    e1 = np.asarray(e1).astype(np.int64)
    out = []
    for c in range(ncores):
        lo = c * npc
        sel = (e0 >= lo) & (e0 < lo + npc)
        ld = e0[sel] - lo
        sr = e1[sel]
        order = np.argsort(ld, kind="stable")
        out.append((ld[order], sr[order]))
    return out


def prep_stream_A(x, per_core, B):
    """Host-gathered layer-A message stream. Per-block tile counts TA[b]
    (max over cores, SPMD-uniform); DRAM padded to TAm per block but only
    TA[b] tiles are transferred/aggregated."""
    blk = []
    TA = [1] * B
    for ld, sr in per_core:
        bid = ld // P
        cnt = np.bincount(bid.astype(np.int64), minlength=B)
        for b in range(B):
            TA[b] = max(TA[b], int(cdiv(max(int(cnt[b]), 1), P)))
        blk.append((ld, sr, bid))
    TAm = max(TA)
    offSA = [0]
    for b in range(B):
        offSA.append(offSA[-1] + TA[b])
    H = x.shape[1]
    out = []
    for ld, sr, bid in blk:
        stream = np.zeros((B, TAm * P, H), NPF16)
        slots = np.full((P, offSA[-1]), 300.0, NPF16)
        for b in range(B):
            m = bid == b
            srcs = sr[m]
            slts = (ld[m] % P).astype(np.float32)
            n = len(srcs)
            stream[b, :n] = x[srcs].astype(NPF16)
            ps = np.full(TA[b] * P, 300.0, np.float32)
            ps[:n] = slts
            slots[:, offSA[b] : offSA[b] + TA[b]] = ps.reshape(TA[b], P).T.astype(NPF16)
        stream = np.ascontiguousarray(
            stream.reshape(B, TAm, P, H).transpose(0, 2, 1, 3)
        ).reshape(B * P, TAm * H)
        out.append(dict(stream=stream, slots=slots))
    return TA, offSA, out


def prep_gather_B(per_core, B, npc, tb_lo, tb_hi):
    """Layer-B edge prep for NT source-range tables.

    Per (dest block b, table k): C[b][k] = cross-core max edge count (shared,
    baked; each core pads its shortfall with idx=0 / slot=300 sentinel rows).
    Per table, the block cells concatenate into one stream that is cut into
    exact ROWCAP-row gather calls (blocks may straddle a call boundary; only
    each table's last call is tile-padded). All rows are valid indices -- HW
    desc-gen charges ~2.4x for -1 skip rows, so dummy row-0 gathers are
    cheaper than skip markers.

    Returns per-table call lists, per-block fragment specs, and per-core
    idx/slot arrays.
    """
    NT = len(tb_lo)
    ncores = len(per_core)
    cells = [[[None] * B for _ in range(NT)] for _ in range(ncores)]
    C = np.zeros((B, NT), np.int64)
    for c, (ld, sr) in enumerate(per_core):
        bid = ld // P
        slot = ld % P
        own_c = sr // npc
        off = sr % npc
        srcblk = off // P
        tbl = np.searchsorted(tb_hi, srcblk, side="right")
        row = own_c * ((np.array(tb_hi) - np.array(tb_lo))[tbl] * P) + (
            off - np.array(tb_lo)[tbl] * P
        )
        for k in range(NT):
            mk = tbl == k
            for b in range(B):
                m = mk & (bid == b)
                cells[c][k][b] = (row[m], slot[m].astype(np.float32))
                C[b, k] = max(C[b, k], int(m.sum()))
    C = np.maximum(C, 1)

    CT = ROWCAP // P * P  # call row capacity, tile-aligned

    # ---- per-table stream layout
    tabs = []  # per table: dict(R[b], S, ncalls, padded[call], ioff[call])
    for k in range(NT):
        R = np.zeros(B + 1, np.int64)
        for b in range(B):
            R[b + 1] = R[b] + C[b, k]
        S = int(R[B])
        ncalls = cdiv(S, CT)
        padded = [CT] * (ncalls - 1) + [cdiv(S - (ncalls - 1) * CT, P) * P]
        tabs.append(dict(k=k, R=R, S=S, ncalls=ncalls, padded=padded))

    # global call list (table-major) + idx prefix offsets
    calls = []  # (k, local_call, padded, ioff)
    ioff = 0
    for t in tabs:
        t["ci0"] = len(calls)
        for lc in range(t["ncalls"]):
            calls.append((t["k"], lc, t["padded"][lc], ioff))
            ioff += t["padded"][lc] // 16
    ncols_idx = ioff

    # per-block fragment/slab specs; completion call per (k, b)
    # stream tile t of table k -> call t // (CT/P), local tile t % (CT/P)
    TPC = CT // P
    blocks = []  # list over (k, b): dict with sall/chunk info
    soff = 0
    for t in tabs:
        k = t["k"]
        R = t["R"]
        for b in range(B):
            tlo = int(R[b]) // P
            thi = cdiv(int(R[b + 1]), P)
            chunks = [(t["ci0"] + tt // TPC, tt % TPC) for tt in range(tlo, thi)]
            blocks.append(
                dict(
                    k=k, b=b, soff=soff, nt=thi - tlo,
                    chunks=chunks, done_ci=chunks[-1][0],
                )
            )
            soff += thi - tlo
    ncols_slot = soff

    out = []
    for c in range(ncores):
        idx = np.zeros((16, ncols_idx), np.int16)
        slots = np.full((P, ncols_slot), 300.0, np.float32)
        for t in tabs:
            k = t["k"]
            R = t["R"]
            Spad = sum(t["padded"])
            vals = np.zeros(Spad, np.int64)
            slv = np.full(Spad, 300.0, np.float32)
            for b in range(B):
                rows, slts = cells[c][k][b]
                n = len(rows)
                vals[int(R[b]) : int(R[b]) + n] = rows
                slv[int(R[b]) : int(R[b]) + n] = slts
            # idx: per call, wrapped-16
            pos = 0
            for lc in range(t["ncalls"]):
                pd = t["padded"][lc]
                io = calls[t["ci0"] + lc][3]
                idx[:, io : io + pd // 16] = (
                    vals[pos : pos + pd].reshape(pd // 16, 16).T.astype(np.int16)
                )
                pos += pd
        for bs in blocks:
            t = tabs[bs["k"]]
            R = t["R"]
            b = bs["b"]
            tlo = int(R[b]) // P
            # slab over stream rows [tlo*P, (tlo+nt)*P), others sentinel
            Spad = sum(t["padded"])
            slv = np.full(Spad + P, 300.0, np.float32)
            rows, slts = cells[c][bs["k"]][b]
            slv[int(R[b]) : int(R[b]) + len(rows)] = slts
            slab = slv[tlo * P : (tlo + bs["nt"]) * P]
            slots[:, bs["soff"] : bs["soff"] + bs["nt"]] = slab.reshape(
                bs["nt"], P
            ).T
        out.append(dict(idx=np.tile(idx, (8, 1)), slots=slots.astype(NPF16)))

    blockspec = tuple(
        (bs["k"], bs["b"], bs["soff"], bs["nt"], tuple(bs["chunks"]), bs["done_ci"])
        for bs in blocks
    )
    return tuple(calls), blockspec, ncols_idx, ncols_slot, out


def prep_all(inputs, ncores=8):
    x = np.asarray(inputs["x"], np.float32)
    N, H = x.shape
    OUT = inputs["Wout"].shape[0]
    npc = N // ncores
    assert npc * ncores == N
    npad = cdiv(npc, P) * P
    B = npad // P
    assert sum(TB) == B, (TB, B)
    tb_lo, tb_hi = [], []
    acc = 0
    for s in TB:
        tb_lo.append(acc)
        acc += s
        tb_hi.append(acc)
        assert s * P * ncores < 32768  # int16 gather index range

    Wl, W0, W1 = (np.asarray(inputs[k], np.float32) for k in ("Wl", "W0", "W1"))
    bl, b0, b1 = (np.asarray(inputs[k], np.float32) for k in ("bl", "b0", "b1"))
    gamma, beta = np.asarray(inputs["gamma"], np.float32), np.asarray(inputs["beta"], np.float32)
    Wout, bout = np.asarray(inputs["Wout"], np.float32), np.asarray(inputs["bout"], np.float32)

    g1, B1 = gamma[1], beta[1]
    g0, B0 = gamma[0], beta[0]
    assert not np.any(B1), "beta of first-applied layer must be 0 (gather fold)"

    WlT_A = Wl[1].T.astype(NPF16)
    W01T_A = (W0[1] + W1[1]).T.astype(NPF16)
    bias_A = bl[1] + b0[1] + b1[1]
    WlT_B = (g1[:, None] * Wl[0].T).astype(NPF16)
    W0T_B = (g1[:, None] * W0[0].T).astype(NPF16)
    W1T_B = W1[0].T.astype(NPF16)
    bias_B = bl[0] + b0[0] + b1[0] + B1 @ W0[0].T
    WoutT = (g0[:, None] * Wout.T).astype(NPF16)
    bout_e = bout + B0 @ Wout.T

    e2 = np.asarray(inputs["edge_r2"])
    e1e = np.asarray(inputs["edge_r1"])
    pcA = sort_edges_by_dest(e2[0], e2[1], ncores, npc)
    pcB = sort_edges_by_dest(e1e[0], e1e[1], ncores, npc)
    TA, offSA, packA = prep_stream_A(x, pcA, B)
    callspec, blockspec, ncols_idx, ncols_slot, packB = prep_gather_B(
        pcB, B, npc, tb_lo, tb_hi
    )

    TMAX = max(max(TA), max(bs[3] for bs in blockspec))
    iota = np.tile(np.arange(P, dtype=np.float32), (P, TMAX)).astype(NPF16)

    cfg = dict(
        N=N, H=H, OUT=OUT, npc=npc, npad=npad, B=B,
        tb_lo=tuple(tb_lo), tb_hi=tuple(tb_hi),
        T_A=tuple(TA), offSA=tuple(offSA),
        callspec=callspec, blockspec=blockspec,
        ncols_idx=ncols_idx, ncols_slot=ncols_slot,
        TMAX=TMAX, ncores=ncores,
        has_bias_A=bool(np.any(bias_A)), has_bias_B=bool(np.any(bias_B)),
        has_bout=bool(np.any(bout_e)),
    )

    in_maps = []
    for c in range(ncores):
        xT_own = np.zeros((H, npad), np.float32)
        xT_own[:, :npc] = x[c * npc : (c + 1) * npc].T
        m = dict(
            gA_stream=packA[c]["stream"], slotA=packA[c]["slots"],
            xT_own=xT_own.astype(NPF16),
            idxB=packB[c]["idx"], slotB=packB[c]["slots"],
            iota=iota,
            WlT_A=WlT_A, W01T_A=W01T_A,
            WlT_B=WlT_B, W0T_B=W0T_B, W1T_B=W1T_B, WoutT=WoutT,
            bias_A=bias_A.reshape(1, H), bias_B=bias_B.reshape(1, H),
            bout_e=bout_e.reshape(1, OUT),
        )
        in_maps.append(m)
    return cfg, in_maps


# ---------------------------------------------------------------- device build

def build_nc(cfg):
    H, OUT, npad, B = cfg["H"], cfg["OUT"], cfg["npad"], cfg["B"]
    TA = cfg["T_A"]
    offSA = cfg["offSA"]
    TAm = max(TA)
    tb_lo, tb_hi = cfg["tb_lo"], cfg["tb_hi"]
    NT = len(tb_lo)
    callspec = cfg["callspec"]
    ncores = cfg["ncores"]
    KH = H // P
    CALL_TILES = max(cs[2] for cs in callspec) // P  # padded tiles per call

    nc = bacc.Bacc(
        "TRN2", target_bir_lowering=False, debug=False, num_devices=ncores,
    )

    def din(name, shape, dt=F16):
        return nc.dram_tensor(name, shape, dt, kind="ExternalInput")

    gA_stream = din("gA_stream", [B * P, TAm * H])
    slotA = din("slotA", [P, offSA[-1]])
    xT_own = din("xT_own", [H, npad])
    idxB = din("idxB", [P, cfg["ncols_idx"]], I16)
    slotB = din("slotB", [P, cfg["ncols_slot"]])
    iota = din("iota", [P, cfg["TMAX"] * P])
    WlT_A = din("WlT_A", [H, H])
    W01T_A = din("W01T_A", [H, H])
    WlT_B = din("WlT_B", [H, H])
    W0T_B = din("W0T_B", [H, H])
    W1T_B = din("W1T_B", [H, H])
    WoutT = din("WoutT", [H, OUT])
    bias_A = din("bias_A", [1, H], F32)
    bias_B = din("bias_B", [1, H], F32)
    bout_e = din("bout_e", [1, OUT], F32)

    blockspec = cfg["blockspec"]
    hk = [(tb_hi[k] - tb_lo[k]) * P for k in range(NT)]
    n1t = [nc.dram_tensor(f"n1_{k}", [hk[k], H], F16) for k in range(NT)]
    tbl = [
        nc.dram_tensor(f"tbl_{k}", [ncores * hk[k], H], F16, addr_space="Shared")
        for k in range(NT)
    ]
    out_own = nc.dram_tensor("out_own", [npad, OUT], F32, kind="ExternalOutput")

    with tile.TileContext(nc) as tc:
        nc.gpsimd.load_library(library_config.mlp)
        with ExitStack() as ctx:
            const = ctx.enter_context(tc.tile_pool(name="const", bufs=1))
            idxp = ctx.enter_context(tc.tile_pool(name="idxp", bufs=1))
            gpoolA = ctx.enter_context(tc.tile_pool(name="gpoolA", bufs=4))
            gpoolB = ctx.enter_context(tc.tile_pool(name="gpoolB", bufs=GPOOL_BUFS))
            sall = ctx.enter_context(tc.tile_pool(name="sall", bufs=2))
            work = ctx.enter_context(tc.tile_pool(name="work", bufs=2))
            ntp = ctx.enter_context(tc.tile_pool(name="ntp", bufs=2))
            stat = ctx.enter_context(tc.tile_pool(name="stat", bufs=3))
            aps = ctx.enter_context(tc.tile_pool(name="aps", bufs=2, space="PSUM"))
            zps = ctx.enter_context(tc.tile_pool(name="zps", bufs=2, space="PSUM"))
            tps = ctx.enter_context(tc.tile_pool(name="tps", bufs=2, space="PSUM"))
            ops = ctx.enter_context(tc.tile_pool(name="ops", bufs=2, space="PSUM"))

            # ---- gather pool buffers, memset once (gpsimd is idle pre-AG0;
            # unwritten tail rows would otherwise be NaN-capable garbage)
            gtiles = []
            for i in range(GPOOL_BUFS):
                g = gpoolB.tile([P, CALL_TILES, H], F16, tag="gB")
                nc.gpsimd.memset(g[:], 0.0)
                gtiles.append(g)

            # ---- constants / persistent tables (order matters: block 0's
            # aggregation needs iota+slotA+stream first; weights only at z)
            iota_t = const.tile([P, cfg["TMAX"] * P], F16)
            nc.sync.dma_start(iota_t[:], iota[:])
            ident = const.tile([P, P], F16)
            make_identity(nc, ident[:])
            eps_col = const.tile([P, 1], F32)
            nc.vector.memset(eps_col[:], EPS)

            slotA_t = idxp.tile(list(slotA.shape), F16, tag="slotA_sb")
            nc.sync.dma_start(slotA_t[:], slotA[:])

            def load_w(t, KN):
                w = const.tile([P, KH, KN], F16, tag=t.name + "_sb")
                nc.sync.dma_start(w[:], t[:].rearrange("(k p) n -> p k n", p=P))
                return w

            wlA = load_w(WlT_A, H)
            w01A = load_w(W01T_A, H)
            wlB = load_w(WlT_B, H)
            w0B = load_w(W0T_B, H)
            w1B = load_w(W1T_B, H)
            wout = load_w(WoutT, OUT)
            if cfg["has_bias_A"]:
                biasA_t = const.tile([1, H], F32)
                nc.sync.dma_start(biasA_t[:], bias_A[:])
            else:
                biasA_t = None
            if cfg["has_bias_B"]:
                biasB_t = const.tile([1, H], F32)
                nc.sync.dma_start(biasB_t[:], bias_B[:])
            else:
                biasB_t = None
            if cfg["has_bout"]:
                bout_t = const.tile([1, OUT], F32)
                nc.sync.dma_start(bout_t[:], bout_e[:])
            else:
                bout_t = None

            def load_flat(t, dt, eng):
                s = idxp.tile(list(t.shape), dt, tag=t.name + "_sb")
                eng.dma_start(s[:], t[:])
                return s

            # layer-B slot table isn't needed until the first gather
            # consumption (~140us in) -- keep it off the startup sync queue
            slotB_t = load_flat(slotB, F16, nc.scalar)
            # gather idx tiles are tiny (128B/partition/call): JIT-load each
            # call's slice a few calls ahead instead of holding all 8.3KB
            idxjit = ctx.enter_context(tc.tile_pool(name="idxjit", bufs=IDX_AHEAD + 3))
            idx_tile = {}

            def idx_load(ci):
                k, lc, padded, ioff = callspec[ci]
                it = idxjit.tile([P, ROWCAP // 16], I16, tag="idxt")
                nc.scalar.dma_start(
                    it[:, 0 : padded // 16], idxB[:, ioff : ioff + padded // 16]
                )
                idx_tile[ci] = it

            # Persistent xT table [feat(p) x (B, KH) x dest]. Chunk 0 on the
            # sync queue (block 0's z needs it); the rest via scalar HWDGE so
            # startup DMA doesn't delay the first stream loads.
            xT_tab = const.tile([P, B, KH, P], F16)
            XCH = cdiv(B, 7)
            for ci in range(XCH):
                b0 = ci * 7
                b1 = min(B, (ci + 1) * 7)
                eng = nc.sync if ci == 0 else nc.scalar
                for k in range(KH):
                    eng.dma_start(
                        xT_tab[:, b0:b1, k, :],
                        xT_own[k * P : (k + 1) * P, b0 * P : b1 * P].rearrange(
                            "p (b d) -> p b d", d=P
                        ),
                    )
            # Persistent transposed layer-A output [feat(p) x (B,KH) x dest].
            n1T_tab = const.tile([P, B, KH, P], F16)
            # Layer-B per-block aggregate accumulator (f16, across table phases)
            aggSB = const.tile([P, B, KH, P], F16)

            # ---------------- shared per-block pieces ----------------

            def build_sall(slot_t, base, nt, tag):
                s = sall.tile([P, cfg["TMAX"], P], F16, tag=tag)
                nc.vector.tensor_tensor(
                    out=s[:, 0:nt, :],
                    in0=slot_t[:, base : base + nt].to_broadcast([P, nt, P])[:],
                    in1=iota_t[:, 0 : nt * P].rearrange("p (t d) -> p t d", t=nt),
                    op=mybir.AluOpType.is_equal,
                )
                return s

            def edge_mms(chunks, s_t):
                """aggT accumulation: agg[:, h, :] += G_half.T @ S per edge tile.
                Sequential accumulation groups only (interleaved groups
                corrupt PSUM)."""
                agg = aps.tile([P, KH, P], F32, tag="agg", space="PSUM")
                nt = len(chunks)
                for h in range(KH):
                    for i, (gt, ch) in enumerate(chunks):
                        nc.tensor.matmul(
                            agg[:, h, :],
                            lhsT=gt[:, ch, h * P : (h + 1) * P],
                            rhs=s_t[:, i, :],
                            start=(i == 0), stop=(i == nt - 1),
                        )
                return agg

            def z_part(agg_lhs, terms, wl, tag):
                """z matmuls from f16 lhsT tiles. Returns z PSUM tile."""
                z = zps.tile([P, H], F32, tag="z", space="PSUM")
                mats = [(agg_lhs, None, wl)] + terms
                mm = [(t, b_, w, k) for (t, b_, w) in mats for k in range(KH)]
                for i, (t, b_, w, k) in enumerate(mm):
                    lhs = t[:, k, :] if b_ is None else t[:, b_, k, :]
                    nc.tensor.matmul(
                        z[:], lhsT=lhs, rhs=w[:, k, :],
                        start=(i == 0), stop=(i == len(mm) - 1),
                    )
                return z

            def ln_part(z, bias_t):
                """relu + LN stats + normalized n_t [P,H] f16."""
                zr = work.tile([P, H], F32, tag="zr")
                s1 = stat.tile([P, 1], F32, tag="s1")
                if bias_t is not None:
                    zb = work.tile([P, H], F32, tag="zb")
                    nc.vector.tensor_tensor(
                        out=zb[:], in0=z[:], in1=bias_t[:].to_broadcast([P, H])[:],
                        op=mybir.AluOpType.add,
                    )
                    zsrc = zb
                else:
                    zsrc = z
                nc.scalar.activation(
                    zr[:], zsrc[:], mybir.ActivationFunctionType.Relu, accum_out=s1[:],
                )
                sq = work.tile([P, H], F32, tag="sq")
                s2 = stat.tile([P, 1], F32, tag="s2")
                nc.scalar.activation(
                    sq[:], zr[:], mybir.ActivationFunctionType.Square, accum_out=s2[:],
                )
                mu = stat.tile([P, 1], F32, tag="mu")
                nc.vector.tensor_scalar_mul(mu[:], s1[:], 1.0 / H)
                ex2 = stat.tile([P, 1], F32, tag="ex2")
                nc.vector.tensor_scalar_mul(ex2[:], s2[:], 1.0 / H)
                mu2 = stat.tile([P, 1], F32, tag="mu2")
                nc.vector.tensor_tensor(out=mu2[:], in0=mu[:], in1=mu[:], op=mybir.AluOpType.mult)
                var = stat.tile([P, 1], F32, tag="var")
                nc.vector.tensor_tensor(out=var[:], in0=ex2[:], in1=mu2[:], op=mybir.AluOpType.subtract)
                std = stat.tile([P, 1], F32, tag="std")
                nc.scalar.activation(
                    std[:], var[:], mybir.ActivationFunctionType.Sqrt, bias=eps_col[:, 0:1],
                )
                rstd = stat.tile([P, 1], F32, tag="rstd")
                nc.vector.reciprocal(rstd[:], std[:])
                nmr = stat.tile([P, 1], F32, tag="nmr")
                nc.vector.scalar_tensor_tensor(
                    out=nmr[:], in0=mu[:], scalar=-1.0, in1=rstd[:],
                    op0=mybir.AluOpType.mult, op1=mybir.AluOpType.mult,
                )
                n_t = ntp.tile([P, H], F16, tag="n_t")
                nc.vector.tensor_scalar(
                    out=n_t[:], in0=zr[:], scalar1=rstd[:, 0:1], scalar2=nmr[:, 0:1],
                    op0=mybir.AluOpType.mult, op1=mybir.AluOpType.add,
                )
                return n_t

            def transpose_pair(n_t, tag="tp"):
                """PE-transpose n_t [dest, H] into [feat(p), KH, dest] PSUM pair."""
                tp = tps.tile([P, KH, P], F16, tag="tp", space="PSUM")
                for k in range(KH):
                    nc.tensor.transpose(tp[:, k, :], n_t[:, k * P : (k + 1) * P], ident[:])
                return tp

            # ---------------- layer A (pipelined) ----------------

            def stream_load(b):
                g = gpoolA.tile([P, TAm, H], F16, tag="gA")
                nc.sync.dma_start(
                    g[:, 0 : TA[b], :].rearrange("p t f -> p (t f)"),
                    gA_stream[b * P : (b + 1) * P, 0 : TA[b] * H],
                )
                return g

            def n1_write(b, n_t):
                k = 0
                while b >= tb_hi[k]:
                    k += 1
                bb = b - tb_lo[k]
                nc.sync.dma_start(n1t[k][bb * P : (bb + 1) * P, :], n_t[:])

            gA = {}
            for b in range(min(3, B)):
                gA[b] = stream_load(b)
            sA = {0: build_sall(slotA_t, 0, TA[0], "sA")}
            aggs = {0: edge_mms([(gA[0], t) for t in range(TA[0])], sA[0])}
            nts = {}
            for b in range(B):
                if b + 1 < B:
                    if b + 3 < B:
                        gA[b + 3] = stream_load(b + 3)
                    sA[b + 1] = build_sall(
                        slotA_t, offSA[b + 1], TA[b + 1], "sA")
                    aggs[b + 1] = edge_mms(
                        [(gA[b + 1], t) for t in range(TA[b + 1])], sA[b + 1]
                    )
                    gA.pop(b, None); sA.pop(b, None)
                agg = aggs.pop(b)
                aT = work.tile([P, KH, P], F16, tag="aT")
                nc.scalar.activation(aT[:], agg[:], mybir.ActivationFunctionType.Copy)
                z = z_part(aT, [(xT_tab, b, w01A)], wlA, "A")
                if b >= 1:
                    tp = transpose_pair(nts[b - 1], "tpA")
                    nc.scalar.activation(
                        n1T_tab[:, b - 1, :, :], tp[:],
                        mybir.ActivationFunctionType.Copy)
                    n1_write(b - 1, nts.pop(b - 1))
                nts[b] = ln_part(z, biasA_t)
            tp = transpose_pair(nts[B - 1], "tpA")
            nc.scalar.activation(
                n1T_tab[:, B - 1, :, :], tp[:],
                mybir.ActivationFunctionType.Copy)
            n1_write(B - 1, nts.pop(B - 1))

            # ---------------- gather + AG schedule on the gpsimd queue ------
            def do_ag(k):
                nc.gpsimd.collective_compute(
                    "AllGather", mybir.AluOpType.bypass,
                    replica_groups=[list(range(ncores))],
                    ins=[n1t[k][:].opt()], outs=[tbl[k][:].opt()],
                )

            def do_gather(ci, g):
                k, lc, padded, ioff = callspec[ci]
                it = idx_tile.pop(ci)
                nc.gpsimd.dma_gather(
                    g[:, 0 : padded // P, :], tbl[k][:],
                    it[:, 0 : padded // 16],
                    padded, padded, H,
                )

            # consumer: per-block aggregation over (possibly two) call tiles,
            # accumulate into aggSB; table NT-1 triggers z/LN/out.
            call_tile = {}

            def consume_block(bs):
                k, b, soff, nt, chunks, done_ci = bs
                s_t = build_sall(slotB_t, soff, nt, "sB")
                agg = edge_mms(
                    [(call_tile[cci], t) for cci, t in chunks], s_t
                )
                if k == 0:
                    nc.vector.tensor_copy(aggSB[:, b, :, :], agg[:])
                else:
                    nc.vector.tensor_tensor(
                        out=aggSB[:, b, :, :], in0=aggSB[:, b, :, :],
                        in1=agg[:], op=mybir.AluOpType.add,
                    )
                if k == NT - 1:
                    finish_block(b)

            def finish_block(b):
                aT = work.tile([P, KH, P], F16, tag="aTB")
                nc.vector.tensor_copy(aT[:], aggSB[:, b, :, :])
                z = z_part(aT, [(n1T_tab, b, w0B), (xT_tab, b, w1B)], wlB, "B")
                n_t = ln_part(z, biasB_t)
                tp = transpose_pair(n_t, "tpB")
                n2T = work.tile([P, KH, P], F16, tag="n2T")
                nc.scalar.activation(n2T[:], tp[:], mybir.ActivationFunctionType.Copy)
                op = ops.tile([P, OUT], F32, tag="op", space="PSUM")
                for k in range(KH):
                    nc.tensor.matmul(
                        op[:], lhsT=n2T[:, k, :], rhs=wout[:, k, :],
                        start=(k == 0), stop=(k == KH - 1),
                    )
                ot = work.tile([P, OUT], F32, tag="ot")
                if bout_t is not None:
                    nc.vector.tensor_tensor(
                        out=ot[:], in0=op[:], in1=bout_t[:].to_broadcast([P, OUT])[:],
                        op=mybir.AluOpType.add,
                    )
                else:
                    nc.scalar.activation(
                        ot[:], op[:], mybir.ActivationFunctionType.Copy)
                nc.sync.dma_start(out_own[b * P : (b + 1) * P, :], ot[:])

            # emit schedule: AG0 first, then gather calls; AG k+1 dispatched
            # AGOFF[k] calls into table k's stream. The last MIX_T2 calls of
            # table NT-2 are deferred and interleaved 1:2 with table NT-1's
            # calls so per-block completions (z/LN/out, ~3.5us PE each) spread
            # over the whole post-AG3 window instead of bunching.
            starts = {}
            for ci, cs in enumerate(callspec):
                starts.setdefault(cs[0], ci)
            agpos = {k + 1: starts[k] + AGOFF[k] for k in range(NT - 1)}
            t2_calls = [ci for ci, cs in enumerate(callspec) if cs[0] == NT - 2]
            t3_calls = [ci for ci, cs in enumerate(callspec) if cs[0] == NT - 1]
            defer = t2_calls[len(t2_calls) - MIX_T2 :] if MIX_T2 else []
            dset = set(defer) | set(t3_calls)
            prefix = [ci for ci in range(len(callspec)) if ci not in dset]
            inter = []
            di = ti = 0
            nd, nt3 = len(defer), len(t3_calls)
            while di < nd or ti < nt3:
                # proportional weave: keep di/nd ~ ti/nt3, t2 slightly ahead
                if di < nd and (ti >= nt3 or di * nt3 <= ti * nd):
                    inter.append(defer[di]); di += 1
                elif ti < nt3:
                    inter.append(t3_calls[ti]); ti += 1
            order = prefix + inter
            emit_pos = {ci: e for e, ci in enumerate(order)}
            consume_at = {}
            for bs in blockspec:
                last = max(emit_pos[cci] for cci, _ in bs[4])
                consume_at.setdefault(last, []).append(bs)
            do_ag(0)
            for e in range(min(IDX_AHEAD, len(order))):
                idx_load(order[e])
            for e, ci in enumerate(order):
                for k, pos in agpos.items():
                    if pos == ci:
                        do_ag(k)
                call_tile[ci] = gtiles[e % GPOOL_BUFS]
                do_gather(ci, call_tile[ci])
                if e + IDX_AHEAD < len(order):
                    idx_load(order[e + IDX_AHEAD])
                for bs in consume_at.get(e, ()):
                    consume_block(bs)

    nc.compile()
    return nc


def run(inputs, ncores=8, nc_cache={}, trace=False, tmpdir=None):
    cfg, in_maps = prep_all(inputs, ncores)
    key = tuple(sorted((k, str(v)) for k, v in cfg.items()))
    if key not in nc_cache:
        nc_cache[key] = build_nc(cfg)
    nc = nc_cache[key]
    res = run_bass_kernel_spmd(
        nc, in_maps, core_ids=list(range(ncores)), trace=trace, tmpdir=tmpdir
    )
    npc = cfg["npc"]
    out = np.concatenate(
        [res.results[c]["out_own"][:npc] for c in range(ncores)], axis=0
    )
    return (out, res) if trace else out


def kernel(**inputs):
    """Full-input entry point: shards across the 8 NeuronCores internally and
    returns the full [N, OUT] float32 output."""
    return np.ascontiguousarray(run(inputs, 8).astype(np.float32))


# revision 29
# speedup vs baseline: 1.2011x; 1.2011x over previous
"""MetaPathGNN Trainium kernel v8 (~796us best, vs v7 927us).

Layer A aggregates a host-pregathered x message stream (one-hot dest-slot
matmuls); its output is published in 4 source-range tables TB=(6,10,16,17)
blocks, each AllGather'd as soon as layer A finishes the range, so layer-B
dma_gathers start at ~135us instead of ~241us. Layer B gathers per-edge rows
in exact 1024-row calls (dest blocks straddle call boundaries; per-(block,
table) counts padded to the cross-core max with idx-0/slot-300 sentinel
rows), accumulates per-block aggregates in an SBUF f16 accumulator across
the 4 table phases, and runs z/LN/out at each block's last fragment. The
last MIX_T2 table-2 calls are woven 1:~2 into table-3's phase so the
~3.5us-PE finish work per block spreads over the post-AG3 window.

Measured hardware walls (don't re-learn these):
- dma_gather (SWDGE): desc-gen serializes on the Pool/Q7 engine at
  ~0.7-1.1us/call + ~7.8ns per VALID row; a -1 (skip) index row costs
  ~19ns - WORSE than gathering row 0, so pad with idx 0, never -1. The
  DMA-transfer side (~3.2us/call spread over 16 engines) is not the
  bottleneck. Hard ~1024-row cap per call; 2 SWDGE queues do NOT
  parallelize desc-gen.
- PE runs at HALF clock most of the time (HAM util-limit 4/8 ~65% of the
  span): 128-free matmul ~215-300ns, 256-free ~310-430ns. 120 back-to-back
  warm-up matmuls do NOT unlock full clock. Flipping edge aggregation to
  S-as-lhsT (one 256-free matmul/tile instead of two 128-free) LOSES ~180us:
  it lengthens the serial finish chain (extra transposes + scalar copies).
- AllGather (RDH): 60-200GB/s alg-bw depending on concurrent DMA; first
  collective pays ~11.5us trigger latency. Warm-up dummy collectives fail
  the BIR verifier (strided/tiny APs) or delay n1 writes (anti-deps). The
  CC dispatch sem-waits in the in-order gpsimd queue: place AG k at AGOFF[k]
  calls into table k-1's stream.
- Consumer matmuls queue behind ALL layer-A PE work (in-order PE queue), so
  gathers stall on pool-buffer reuse until layer-A drains (~300us):
  GPOOL_BUFS must cover (drain - first gather)/call period.
- Interleaved PSUM accumulation groups in one bank CORRUPT results; keep
  accumulation groups sequential.
- Per-partition-contiguous DRAM stream layout ([B*P, T_A*H]) gives 6KB
  descriptors (~line rate); naive layout caps at ~190GB/s.
- fp8 anywhere in the data path gives ~2% output error (gate is 2e-2) -
  e4m3 rms ~3.6% on N(0,1) data; int16 gather indices cap tables at
  32768 rows (blocks*128*8cores).
- Chip-level thermal throttle: back-to-back runs can measure +15-20%
  (gather calls 8.6 -> 10.3us); cool-down before timing.
"""

import numpy as np
from contextlib import ExitStack

import concourse.bass as bass
import concourse.tile as tile
from concourse import bacc, mybir, library_config
from concourse.bass_utils import run_bass_kernel_spmd
from concourse.masks import make_identity

P = 128
F32 = mybir.dt.float32
F16 = mybir.dt.float16
I16 = mybir.dt.int16
NPF16 = np.float16
EPS = 1e-5

# ---- v8 tuning knobs
TB = (6, 10, 16, 17)      # blocks per layer-A output table (sum must be B=49)
ROWCAP = 1024             # rows per gather call (hard SWDGE carveout cap)
# AG k+1 dispatched after this many gather calls of table k have issued
# (must be late enough that layer A produced table k+1's blocks, early
# enough that AG k+1 completes before table k's gather stream drains).
AGOFF = (1, 5, 1)
MIX_T2 = 11  # table-2 calls deferred into table-3's phase (spreads finish-block PE)
GPOOL_BUFS = 17           # gather pool depth (calls in flight)


def cdiv(a, b):
    return (a + b - 1) // b


# ---------------------------------------------------------------- host prep

def sort_edges_by_dest(e0, e1, ncores, npc):
    """Per core: edge (local_dest, src) arrays sorted by local dest."""
    e0 = np.asarray(e0).astype(np.int64)
    e1 = np.asarray(e1).astype(np.int64)
    out = []
    for c in range(ncores):
        lo = c * npc
        sel = (e0 >= lo) & (e0 < lo + npc)
        ld = e0[sel] - lo
        sr = e1[sel]
        order = np.argsort(ld, kind="stable")
        out.append((ld[order], sr[order]))
    return out


def prep_stream_A(x, per_core, B):
    """Host-gathered layer-A message stream. Per-block tile counts TA[b]
    (max over cores, SPMD-uniform); DRAM padded to TAm per block but only
    TA[b] tiles are transferred/aggregated."""
    blk = []
    TA = [1] * B
    for ld, sr in per_core:
        bid = ld // P
        cnt = np.bincount(bid.astype(np.int64), minlength=B)
        for b in range(B):
            TA[b] = max(TA[b], int(cdiv(max(int(cnt[b]), 1), P)))
        blk.append((ld, sr, bid))
    TAm = max(TA)
    offSA = [0]
    for b in range(B):
        offSA.append(offSA[-1] + TA[b])
    H = x.shape[1]
    out = []
    for ld, sr, bid in blk:
        stream = np.zeros((B, TAm * P, H), NPF16)
        slots = np.full((P, offSA[-1]), 300.0, NPF16)
        for b in range(B):
            m = bid == b
            srcs = sr[m]
            slts = (ld[m] % P).astype(np.float32)
            n = len(srcs)
            stream[b, :n] = x[srcs].astype(NPF16)
            ps = np.full(TA[b] * P, 300.0, np.float32)
            ps[:n] = slts
            slots[:, offSA[b] : offSA[b] + TA[b]] = ps.reshape(TA[b], P).T.astype(NPF16)
        stream = np.ascontiguousarray(
            stream.reshape(B, TAm, P, H).transpose(0, 2, 1, 3)
        ).reshape(B * P, TAm * H)
        out.append(dict(stream=stream, slots=slots))
    return TA, offSA, out


def prep_gather_B(per_core, B, npc, tb_lo, tb_hi):
    """Layer-B edge prep for NT source-range tables.

    Per (dest block b, table k): C[b][k] = cross-core max edge count (shared,
    baked; each core pads its shortfall with idx=0 / slot=300 sentinel rows).
    Per table, the block cells concatenate into one stream that is cut into
    exact ROWCAP-row gather calls (blocks may straddle a call boundary; only
    each table's last call is tile-padded). All rows are valid indices -- HW
    desc-gen charges ~2.4x for -1 skip rows, so dummy row-0 gathers are
    cheaper than skip markers.

    Returns per-table call lists, per-block fragment specs, and per-core
    idx/slot arrays.
    """
    NT = len(tb_lo)
    ncores = len(per_core)
    cells = [[[None] * B for _ in range(NT)] for _ in range(ncores)]
    C = np.zeros((B, NT), np.int64)
    for c, (ld, sr) in enumerate(per_core):
        bid = ld // P
        slot = ld % P
        own_c = sr // npc
        off = sr % npc
        srcblk = off // P
        tbl = np.searchsorted(tb_hi, srcblk, side="right")
        row = own_c * ((np.array(tb_hi) - np.array(tb_lo))[tbl] * P) + (
            off - np.array(tb_lo)[tbl] * P
        )
        for k in range(NT):
            mk = tbl == k
            for b in range(B):
                m = mk & (bid == b)
                cells[c][k][b] = (row[m], slot[m].astype(np.float32))
                C[b, k] = max(C[b, k], int(m.sum()))
    C = np.maximum(C, 1)

    CT = ROWCAP // P * P  # call row capacity, tile-aligned

    # ---- per-table stream layout
    tabs = []  # per table: dict(R[b], S, ncalls, padded[call], ioff[call])
    for k in range(NT):
        R = np.zeros(B + 1, np.int64)
        for b in range(B):
            R[b + 1] = R[b] + C[b, k]
        S = int(R[B])
        ncalls = cdiv(S, CT)
        padded = [CT] * (ncalls - 1) + [cdiv(S - (ncalls - 1) * CT, P) * P]
        tabs.append(dict(k=k, R=R, S=S, ncalls=ncalls, padded=padded))

    # global call list (table-major) + idx prefix offsets
    calls = []  # (k, local_call, padded, ioff)
    ioff = 0
    for t in tabs:
        t["ci0"] = len(calls)
        for lc in range(t["ncalls"]):
            calls.append((t["k"], lc, t["padded"][lc], ioff))
            ioff += t["padded"][lc] // 16
    ncols_idx = ioff

    # per-block fragment/slab specs; completion call per (k, b)
    # stream tile t of table k -> call t // (CT/P), local tile t % (CT/P)
    TPC = CT // P
    blocks = []  # list over (k, b): dict with sall/chunk info
    soff = 0
    for t in tabs:
        k = t["k"]
        R = t["R"]
        for b in range(B):
            tlo = int(R[b]) // P
            thi = cdiv(int(R[b + 1]), P)
            chunks = [(t["ci0"] + tt // TPC, tt % TPC) for tt in range(tlo, thi)]
            blocks.append(
                dict(
                    k=k, b=b, soff=soff, nt=thi - tlo,
                    chunks=chunks, done_ci=chunks[-1][0],
                )
            )
            soff += thi - tlo
    ncols_slot = soff

    out = []
    for c in range(ncores):
        idx = np.zeros((16, ncols_idx), np.int16)
        slots = np.full((P, ncols_slot), 300.0, np.float32)
        for t in tabs:
            k = t["k"]
            R = t["R"]
            Spad = sum(t["padded"])
            vals = np.zeros(Spad, np.int64)
            slv = np.full(Spad, 300.0, np.float32)
            for b in range(B):
                rows, slts = cells[c][k][b]
                n = len(rows)
                vals[int(R[b]) : int(R[b]) + n] = rows
                slv[int(R[b]) : int(R[b]) + n] = slts
            # idx: per call, wrapped-16
            pos = 0
            for lc in range(t["ncalls"]):
                pd = t["padded"][lc]
                io = calls[t["ci0"] + lc][3]
                idx[:, io : io + pd // 16] = (
                    vals[pos : pos + pd].reshape(pd // 16, 16).T.astype(np.int16)
                )
                pos += pd
        for bs in blocks:
            t = tabs[bs["k"]]
            R = t["R"]
            b = bs["b"]
            tlo = int(R[b]) // P
            # slab over stream rows [tlo*P, (tlo+nt)*P), others sentinel
            Spad = sum(t["padded"])
            slv = np.full(Spad + P, 300.0, np.float32)
            rows, slts = cells[c][bs["k"]][b]
            slv[int(R[b]) : int(R[b]) + len(rows)] = slts
            slab = slv[tlo * P : (tlo + bs["nt"]) * P]
            slots[:, bs["soff"] : bs["soff"] + bs["nt"]] = slab.reshape(
                bs["nt"], P
            ).T
        out.append(dict(idx=np.tile(idx, (8, 1)), slots=slots.astype(NPF16)))

    blockspec = tuple(
        (bs["k"], bs["b"], bs["soff"], bs["nt"], tuple(bs["chunks"]), bs["done_ci"])
        for bs in blocks
    )
    return tuple(calls), blockspec, ncols_idx, ncols_slot, out


def prep_all(inputs, ncores=8):
    x = np.asarray(inputs["x"], np.float32)
    N, H = x.shape
    OUT = inputs["Wout"].shape[0]
    npc = N // ncores
    assert npc * ncores == N
    npad = cdiv(npc, P) * P
    B = npad // P
    assert sum(TB) == B, (TB, B)
    tb_lo, tb_hi = [], []
    acc = 0
    for s in TB:
        tb_lo.append(acc)
        acc += s
        tb_hi.append(acc)
        assert s * P * ncores < 32768  # int16 gather index range

    Wl, W0, W1 = (np.asarray(inputs[k], np.float32) for k in ("Wl", "W0", "W1"))
    bl, b0, b1 = (np.asarray(inputs[k], np.float32) for k in ("bl", "b0", "b1"))
    gamma, beta = np.asarray(inputs["gamma"], np.float32), np.asarray(inputs["beta"], np.float32)
    Wout, bout = np.asarray(inputs["Wout"], np.float32), np.asarray(inputs["bout"], np.float32)

    g1, B1 = gamma[1], beta[1]
    g0, B0 = gamma[0], beta[0]
    assert not np.any(B1), "beta of first-applied layer must be 0 (gather fold)"

    WlT_A = Wl[1].T.astype(NPF16)
    W01T_A = (W0[1] + W1[1]).T.astype(NPF16)
    bias_A = bl[1] + b0[1] + b1[1]
    WlT_B = (g1[:, None] * Wl[0].T).astype(NPF16)
    W0T_B = (g1[:, None] * W0[0].T).astype(NPF16)
    W1T_B = W1[0].T.astype(NPF16)
    bias_B = bl[0] + b0[0] + b1[0] + B1 @ W0[0].T
    WoutT = (g0[:, None] * Wout.T).astype(NPF16)
    bout_e = bout + B0 @ Wout.T

    e2 = np.asarray(inputs["edge_r2"])
    e1e = np.asarray(inputs["edge_r1"])
    pcA = sort_edges_by_dest(e2[0], e2[1], ncores, npc)
    pcB = sort_edges_by_dest(e1e[0], e1e[1], ncores, npc)
    TA, offSA, packA = prep_stream_A(x, pcA, B)
    callspec, blockspec, ncols_idx, ncols_slot, packB = prep_gather_B(
        pcB, B, npc, tb_lo, tb_hi
    )

    TMAX = max(max(TA), max(bs[3] for bs in blockspec))
    iota = np.tile(np.arange(P, dtype=np.float32), (P, TMAX)).astype(NPF16)

    cfg = dict(
        N=N, H=H, OUT=OUT, npc=npc, npad=npad, B=B,
        tb_lo=tuple(tb_lo), tb_hi=tuple(tb_hi),
        T_A=tuple(TA), offSA=tuple(offSA),
        callspec=callspec, blockspec=blockspec,
        ncols_idx=ncols_idx, ncols_slot=ncols_slot,
        TMAX=TMAX, ncores=ncores,
        has_bias_A=bool(np.any(bias_A)), has_bias_B=bool(np.any(bias_B)),
        has_bout=bool(np.any(bout_e)),
    )

    in_maps = []
    for c in range(ncores):
        xT_own = np.zeros((H, npad), np.float32)
        xT_own[:, :npc] = x[c * npc : (c + 1) * npc].T
        m = dict(
            gA_stream=packA[c]["stream"], slotA=packA[c]["slots"],
            xT_own=xT_own.astype(NPF16),
            idxB=packB[c]["idx"], slotB=packB[c]["slots"],
            iota=iota,
            WlT_A=WlT_A, W01T_A=W01T_A,
            WlT_B=WlT_B, W0T_B=W0T_B, W1T_B=W1T_B, WoutT=WoutT,
            bias_A=bias_A.reshape(1, H), bias_B=bias_B.reshape(1, H),
            bout_e=bout_e.reshape(1, OUT),
        )
        in_maps.append(m)
    return cfg, in_maps


# ---------------------------------------------------------------- device build

def build_nc(cfg):
    H, OUT, npad, B = cfg["H"], cfg["OUT"], cfg["npad"], cfg["B"]
    TA = cfg["T_A"]
    offSA = cfg["offSA"]
    TAm = max(TA)
    tb_lo, tb_hi = cfg["tb_lo"], cfg["tb_hi"]
    NT = len(tb_lo)
    callspec = cfg["callspec"]
    ncores = cfg["ncores"]
    KH = H // P
    CALL_TILES = max(cs[2] for cs in callspec) // P  # padded tiles per call

    nc = bacc.Bacc(
        "TRN2", target_bir_lowering=False, debug=False, num_devices=ncores,
    )

    def din(name, shape, dt=F16):
        return nc.dram_tensor(name, shape, dt, kind="ExternalInput")

    gA_stream = din("gA_stream", [B * P, TAm * H])
    slotA = din("slotA", [P, offSA[-1]])
    xT_own = din("xT_own", [H, npad])
    idxB = din("idxB", [P, cfg["ncols_idx"]], I16)
    slotB = din("slotB", [P, cfg["ncols_slot"]])
    iota = din("iota", [P, cfg["TMAX"] * P])
    WlT_A = din("WlT_A", [H, H])
    W01T_A = din("W01T_A", [H, H])
    WlT_B = din("WlT_B", [H, H])
    W0T_B = din("W0T_B", [H, H])
    W1T_B = din("W1T_B", [H, H])
    WoutT = din("WoutT", [H, OUT])
    bias_A = din("bias_A", [1, H], F32)
    bias_B = din("bias_B", [1, H], F32)
    bout_e = din("bout_e", [1, OUT], F32)

    blockspec = cfg["blockspec"]
    hk = [(tb_hi[k] - tb_lo[k]) * P for k in range(NT)]
    n1t = [nc.dram_tensor(f"n1_{k}", [hk[k], H], F16) for k in range(NT)]
    tbl = [
        nc.dram_tensor(f"tbl_{k}", [ncores * hk[k], H], F16, addr_space="Shared")
        for k in range(NT)
    ]
    out_own = nc.dram_tensor("out_own", [npad, OUT], F32, kind="ExternalOutput")

    with tile.TileContext(nc) as tc:
        nc.gpsimd.load_library(library_config.mlp)
        with ExitStack() as ctx:
            const = ctx.enter_context(tc.tile_pool(name="const", bufs=1))
            idxp = ctx.enter_context(tc.tile_pool(name="idxp", bufs=1))
            gpoolA = ctx.enter_context(tc.tile_pool(name="gpoolA", bufs=4))
            gpoolB = ctx.enter_context(tc.tile_pool(name="gpoolB", bufs=GPOOL_BUFS))
            sall = ctx.enter_context(tc.tile_pool(name="sall", bufs=2))
            work = ctx.enter_context(tc.tile_pool(name="work", bufs=2))
            ntp = ctx.enter_context(tc.tile_pool(name="ntp", bufs=2))
            stat = ctx.enter_context(tc.tile_pool(name="stat", bufs=3))
            aps = ctx.enter_context(tc.tile_pool(name="aps", bufs=2, space="PSUM"))
            zps = ctx.enter_context(tc.tile_pool(name="zps", bufs=2, space="PSUM"))
            tps = ctx.enter_context(tc.tile_pool(name="tps", bufs=2, space="PSUM"))
            ops = ctx.enter_context(tc.tile_pool(name="ops", bufs=2, space="PSUM"))

            # ---- gather pool buffers, memset once (gpsimd is idle pre-AG0;
            # unwritten tail rows would otherwise be NaN-capable garbage)
            gtiles = []
            for i in range(GPOOL_BUFS):
                g = gpoolB.tile([P, CALL_TILES, H], F16, tag="gB")
                nc.gpsimd.memset(g[:], 0.0)
                gtiles.append(g)

            # ---- constants / persistent tables (order matters: block 0's
            # aggregation needs iota+slotA+stream first; weights only at z)
            iota_t = const.tile([P, cfg["TMAX"] * P], F16)
            nc.sync.dma_start(iota_t[:], iota[:])
            ident = const.tile([P, P], F16)
            make_identity(nc, ident[:])
            eps_col = const.tile([P, 1], F32)
            nc.vector.memset(eps_col[:], EPS)

            slotA_t = idxp.tile(list(slotA.shape), F16, tag="slotA_sb")
            nc.sync.dma_start(slotA_t[:], slotA[:])

            def load_w(t, KN):
                w = const.tile([P, KH, KN], F16, tag=t.name + "_sb")
                nc.sync.dma_start(w[:], t[:].rearrange("(k p) n -> p k n", p=P))
                return w

            wlA = load_w(WlT_A, H)
            w01A = load_w(W01T_A, H)
            wlB = load_w(WlT_B, H)
            w0B = load_w(W0T_B, H)
            w1B = load_w(W1T_B, H)
            wout = load_w(WoutT, OUT)
            if cfg["has_bias_A"]:
                biasA_t = const.tile([1, H], F32)
                nc.sync.dma_start(biasA_t[:], bias_A[:])
            else:
                biasA_t = None
            if cfg["has_bias_B"]:
                biasB_t = const.tile([1, H], F32)
                nc.sync.dma_start(biasB_t[:], bias_B[:])
            else:
                biasB_t = None
            if cfg["has_bout"]:
                bout_t = const.tile([1, OUT], F32)
                nc.sync.dma_start(bout_t[:], bout_e[:])
            else:
                bout_t = None

            def load_flat(t, dt, eng):
                s = idxp.tile(list(t.shape), dt, tag=t.name + "_sb")
                eng.dma_start(s[:], t[:])
                return s

            # layer-B index/slot tables aren't needed until the first gather
            # consumption (~140us in) -- keep them off the startup sync queue
            idxB_t = load_flat(idxB, I16, nc.scalar)
            slotB_t = load_flat(slotB, F16, nc.scalar)

            # Persistent xT table [feat(p) x (B, KH) x dest]. Chunk 0 on the
            # sync queue (block 0's z needs it); the rest via scalar HWDGE so
            # startup DMA doesn't delay the first stream loads.
            xT_tab = const.tile([P, B, KH, P], F16)
            XCH = cdiv(B, 7)
            for ci in range(XCH):
                b0 = ci * 7
                b1 = min(B, (ci + 1) * 7)
                eng = nc.sync if ci == 0 else nc.scalar
                for k in range(KH):
                    eng.dma_start(
                        xT_tab[:, b0:b1, k, :],
                        xT_own[k * P : (k + 1) * P, b0 * P : b1 * P].rearrange(
                            "p (b d) -> p b d", d=P
                        ),
                    )
            # Persistent transposed layer-A output [feat(p) x (B,KH) x dest].
            n1T_tab = const.tile([P, B, KH, P], F16)
            # Layer-B per-block aggregate accumulator (f16, across table phases)
            aggSB = const.tile([P, B, KH, P], F16)

            # ---------------- shared per-block pieces ----------------

            def build_sall(slot_t, base, nt, tag):
                s = sall.tile([P, cfg["TMAX"], P], F16, tag=tag)
                nc.vector.tensor_tensor(
                    out=s[:, 0:nt, :],
                    in0=slot_t[:, base : base + nt].to_broadcast([P, nt, P])[:],
                    in1=iota_t[:, 0 : nt * P].rearrange("p (t d) -> p t d", t=nt),
                    op=mybir.AluOpType.is_equal,
                )
                return s

            def edge_mms(chunks, s_t):
                """aggT accumulation: agg[:, h, :] += G_half.T @ S per edge tile.
                Sequential accumulation groups only (interleaved groups
                corrupt PSUM)."""
                agg = aps.tile([P, KH, P], F32, tag="agg", space="PSUM")
                nt = len(chunks)
                for h in range(KH):
                    for i, (gt, ch) in enumerate(chunks):
                        nc.tensor.matmul(
                            agg[:, h, :],
                            lhsT=gt[:, ch, h * P : (h + 1) * P],
                            rhs=s_t[:, i, :],
                            start=(i == 0), stop=(i == nt - 1),
                        )
                return agg

            def z_part(agg_lhs, terms, wl, tag):
                """z matmuls from f16 lhsT tiles. Returns z PSUM tile."""
                z = zps.tile([P, H], F32, tag="z", space="PSUM")
                mats = [(agg_lhs, None, wl)] + terms
                mm = [(t, b_, w, k) for (t, b_, w) in mats for k in range(KH)]
                for i, (t, b_, w, k) in enumerate(mm):
                    lhs = t[:, k, :] if b_ is None else t[:, b_, k, :]
                    nc.tensor.matmul(
                        z[:], lhsT=lhs, rhs=w[:, k, :],
                        start=(i == 0), stop=(i == len(mm) - 1),
                    )
                return z

            def ln_part(z, bias_t):
                """relu + LN stats + normalized n_t [P,H] f16."""
                zr = work.tile([P, H], F32, tag="zr")
                s1 = stat.tile([P, 1], F32, tag="s1")
                if bias_t is not None:
                    zb = work.tile([P, H], F32, tag="zb")
                    nc.vector.tensor_tensor(
                        out=zb[:], in0=z[:], in1=bias_t[:].to_broadcast([P, H])[:],
                        op=mybir.AluOpType.add,
                    )
                    zsrc = zb
                else:
                    zsrc = z
                nc.scalar.activation(
                    zr[:], zsrc[:], mybir.ActivationFunctionType.Relu, accum_out=s1[:],
                )
                sq = work.tile([P, H], F32, tag="sq")
                s2 = stat.tile([P, 1], F32, tag="s2")
                nc.scalar.activation(
                    sq[:], zr[:], mybir.ActivationFunctionType.Square, accum_out=s2[:],
                )
                mu = stat.tile([P, 1], F32, tag="mu")
                nc.vector.tensor_scalar_mul(mu[:], s1[:], 1.0 / H)
                ex2 = stat.tile([P, 1], F32, tag="ex2")
                nc.vector.tensor_scalar_mul(ex2[:], s2[:], 1.0 / H)
                mu2 = stat.tile([P, 1], F32, tag="mu2")
                nc.vector.tensor_tensor(out=mu2[:], in0=mu[:], in1=mu[:], op=mybir.AluOpType.mult)
                var = stat.tile([P, 1], F32, tag="var")
                nc.vector.tensor_tensor(out=var[:], in0=ex2[:], in1=mu2[:], op=mybir.AluOpType.subtract)
                std = stat.tile([P, 1], F32, tag="std")
                nc.scalar.activation(
                    std[:], var[:], mybir.ActivationFunctionType.Sqrt, bias=eps_col[:, 0:1],
                )
                rstd = stat.tile([P, 1], F32, tag="rstd")
                nc.vector.reciprocal(rstd[:], std[:])
                nmr = stat.tile([P, 1], F32, tag="nmr")
                nc.vector.scalar_tensor_tensor(
                    out=nmr[:], in0=mu[:], scalar=-1.0, in1=rstd[:],
                    op0=mybir.AluOpType.mult, op1=mybir.AluOpType.mult,
                )
                n_t = ntp.tile([P, H], F16, tag="n_t")
                nc.vector.tensor_scalar(
                    out=n_t[:], in0=zr[:], scalar1=rstd[:, 0:1], scalar2=nmr[:, 0:1],
                    op0=mybir.AluOpType.mult, op1=mybir.AluOpType.add,
                )
                return n_t

            def transpose_pair(n_t, tag="tp"):
                """PE-transpose n_t [dest, H] into [feat(p), KH, dest] PSUM pair."""
                tp = tps.tile([P, KH, P], F16, tag="tp", space="PSUM")
                for k in range(KH):
                    nc.tensor.transpose(tp[:, k, :], n_t[:, k * P : (k + 1) * P], ident[:])
                return tp

            # ---------------- layer A (pipelined) ----------------

            def stream_load(b):
                g = gpoolA.tile([P, TAm, H], F16, tag="gA")
                nc.sync.dma_start(
                    g[:, 0 : TA[b], :].rearrange("p t f -> p (t f)"),
                    gA_stream[b * P : (b + 1) * P, 0 : TA[b] * H],
                )
                return g

            def n1_write(b, n_t):
                k = 0
                while b >= tb_hi[k]:
                    k += 1
                bb = b - tb_lo[k]
                nc.sync.dma_start(n1t[k][bb * P : (bb + 1) * P, :], n_t[:])

            gA = {}
            for b in range(min(3, B)):
                gA[b] = stream_load(b)
            sA = {0: build_sall(slotA_t, 0, TA[0], "sA")}
            aggs = {0: edge_mms([(gA[0], t) for t in range(TA[0])], sA[0])}
            nts = {}
            for b in range(B):
                if b + 1 < B:
                    if b + 3 < B:
                        gA[b + 3] = stream_load(b + 3)
                    sA[b + 1] = build_sall(
                        slotA_t, offSA[b + 1], TA[b + 1], "sA")
                    aggs[b + 1] = edge_mms(
                        [(gA[b + 1], t) for t in range(TA[b + 1])], sA[b + 1]
                    )
                    gA.pop(b, None); sA.pop(b, None)
                agg = aggs.pop(b)
                aT = work.tile([P, KH, P], F16, tag="aT")
                nc.scalar.activation(aT[:], agg[:], mybir.ActivationFunctionType.Copy)
                z = z_part(aT, [(xT_tab, b, w01A)], wlA, "A")
                if b >= 1:
                    tp = transpose_pair(nts[b - 1], "tpA")
                    nc.scalar.activation(
                        n1T_tab[:, b - 1, :, :], tp[:],
                        mybir.ActivationFunctionType.Copy)
                    n1_write(b - 1, nts.pop(b - 1))
                nts[b] = ln_part(z, biasA_t)
            tp = transpose_pair(nts[B - 1], "tpA")
            nc.scalar.activation(
                n1T_tab[:, B - 1, :, :], tp[:],
                mybir.ActivationFunctionType.Copy)
            n1_write(B - 1, nts.pop(B - 1))

            # ---------------- gather + AG schedule on the gpsimd queue ------
            def do_ag(k):
                nc.gpsimd.collective_compute(
                    "AllGather", mybir.AluOpType.bypass,
                    replica_groups=[list(range(ncores))],
                    ins=[n1t[k][:].opt()], outs=[tbl[k][:].opt()],
                )

            def do_gather(ci, g):
                k, lc, padded, ioff = callspec[ci]
                nc.gpsimd.dma_gather(
                    g[:, 0 : padded // P, :], tbl[k][:],
                    idxB_t[:, ioff : ioff + padded // 16],
                    padded, padded, H,
                )

            # consumer: per-block aggregation over (possibly two) call tiles,
            # accumulate into aggSB; table NT-1 triggers z/LN/out.
            call_tile = {}

            def consume_block(bs):
                k, b, soff, nt, chunks, done_ci = bs
                s_t = build_sall(slotB_t, soff, nt, "sB")
                agg = edge_mms(
                    [(call_tile[cci], t) for cci, t in chunks], s_t
                )
                if k == 0:
                    nc.vector.tensor_copy(aggSB[:, b, :, :], agg[:])
                else:
                    nc.vector.tensor_tensor(
                        out=aggSB[:, b, :, :], in0=aggSB[:, b, :, :],
                        in1=agg[:], op=mybir.AluOpType.add,
                    )
                if k == NT - 1:
                    finish_block(b)

            def finish_block(b):
                aT = work.tile([P, KH, P], F16, tag="aTB")
                nc.scalar.activation(
                    aT[:], aggSB[:, b, :, :], mybir.ActivationFunctionType.Copy)
                z = z_part(aT, [(n1T_tab, b, w0B), (xT_tab, b, w1B)], wlB, "B")
                n_t = ln_part(z, biasB_t)
                tp = transpose_pair(n_t, "tpB")
                n2T = work.tile([P, KH, P], F16, tag="n2T")
                nc.scalar.activation(n2T[:], tp[:], mybir.ActivationFunctionType.Copy)
                op = ops.tile([P, OUT], F32, tag="op", space="PSUM")
                for k in range(KH):
                    nc.tensor.matmul(
                        op[:], lhsT=n2T[:, k, :], rhs=wout[:, k, :],
                        start=(k == 0), stop=(k == KH - 1),
                    )
                ot = work.tile([P, OUT], F32, tag="ot")
                if bout_t is not None:
                    nc.vector.tensor_tensor(
                        out=ot[:], in0=op[:], in1=bout_t[:].to_broadcast([P, OUT])[:],
                        op=mybir.AluOpType.add,
                    )
                else:
                    nc.scalar.activation(
                        ot[:], op[:], mybir.ActivationFunctionType.Copy)
                nc.sync.dma_start(out_own[b * P : (b + 1) * P, :], ot[:])

            # emit schedule: AG0 first, then gather calls; AG k+1 dispatched
            # AGOFF[k] calls into table k's stream. The last MIX_T2 calls of
            # table NT-2 are deferred and interleaved 1:2 with table NT-1's
            # calls so per-block completions (z/LN/out, ~3.5us PE each) spread
            # over the whole post-AG3 window instead of bunching.
            starts = {}
            for ci, cs in enumerate(callspec):
                starts.setdefault(cs[0], ci)
            agpos = {k + 1: starts[k] + AGOFF[k] for k in range(NT - 1)}
            t2_calls = [ci for ci, cs in enumerate(callspec) if cs[0] == NT - 2]
            t3_calls = [ci for ci, cs in enumerate(callspec) if cs[0] == NT - 1]
            defer = t2_calls[len(t2_calls) - MIX_T2 :] if MIX_T2 else []
            dset = set(defer) | set(t3_calls)
            prefix = [ci for ci in range(len(callspec)) if ci not in dset]
            inter = []
            di = ti = 0
            nd, nt3 = len(defer), len(t3_calls)
            while di < nd or ti < nt3:
                # proportional weave: keep di/nd ~ ti/nt3, t2 slightly ahead
                if di < nd and (ti >= nt3 or di * nt3 <= ti * nd):
                    inter.append(defer[di]); di += 1
                elif ti < nt3:
                    inter.append(t3_calls[ti]); ti += 1
            order = prefix + inter
            emit_pos = {ci: e for e, ci in enumerate(order)}
            consume_at = {}
            for bs in blockspec:
                last = max(emit_pos[cci] for cci, _ in bs[4])
                consume_at.setdefault(last, []).append(bs)
            do_ag(0)
            for e, ci in enumerate(order):
                for k, pos in agpos.items():
                    if pos == ci:
                        do_ag(k)
                call_tile[ci] = gtiles[e % GPOOL_BUFS]
                do_gather(ci, call_tile[ci])
                for bs in consume_at.get(e, ()):
                    consume_block(bs)

    nc.compile()
    return nc


def run(inputs, ncores=8, nc_cache={}, trace=False, tmpdir=None):
    cfg, in_maps = prep_all(inputs, ncores)
    key = tuple(sorted((k, str(v)) for k, v in cfg.items()))
    if key not in nc_cache:
        nc_cache[key] = build_nc(cfg)
    nc = nc_cache[key]
    res = run_bass_kernel_spmd(
        nc, in_maps, core_ids=list(range(ncores)), trace=trace, tmpdir=tmpdir
    )
    npc = cfg["npc"]
    out = np.concatenate(
        [res.results[c]["out_own"][:npc] for c in range(ncores)], axis=0
    )
    return (out, res) if trace else out


def kernel(**inputs):
    """Full-input entry point: shards across the 8 NeuronCores internally and
    returns the full [N, OUT] float32 output."""
    return np.ascontiguousarray(run(inputs, 8).astype(np.float32))


# revision 31
# speedup vs baseline: 1.2089x; 1.0065x over previous
"""MetaPathGNN Trainium kernel v8 (~796us best, vs v7 927us).

Layer A aggregates a host-pregathered x message stream (one-hot dest-slot
matmuls); its output is published in 4 source-range tables TB=(6,10,16,17)
blocks, each AllGather'd as soon as layer A finishes the range, so layer-B
dma_gathers start at ~135us instead of ~241us. Layer B gathers per-edge rows
in exact 1024-row calls (dest blocks straddle call boundaries; per-(block,
table) counts padded to the cross-core max with idx-0/slot-300 sentinel
rows), accumulates per-block aggregates in an SBUF f16 accumulator across
the 4 table phases, and runs z/LN/out at each block's last fragment. The
last MIX_T2 table-2 calls are woven 1:~2 into table-3's phase so the
~3.5us-PE finish work per block spreads over the post-AG3 window.

Measured hardware walls (don't re-learn these):
- dma_gather (SWDGE): desc-gen serializes on the Pool/Q7 engine at
  ~0.7-1.1us/call + ~7.8ns per VALID row; a -1 (skip) index row costs
  ~19ns - WORSE than gathering row 0, so pad with idx 0, never -1. The
  DMA-transfer side (~3.2us/call spread over 16 engines) is not the
  bottleneck. Hard ~1024-row cap per call; 2 SWDGE queues do NOT
  parallelize desc-gen.
- PE runs at HALF clock most of the time (HAM util-limit 4/8 ~65% of the
  span): 128-free matmul ~215-300ns, 256-free ~310-430ns. 120 back-to-back
  warm-up matmuls do NOT unlock full clock. Flipping edge aggregation to
  S-as-lhsT (one 256-free matmul/tile instead of two 128-free) LOSES ~180us:
  it lengthens the serial finish chain (extra transposes + scalar copies).
- AllGather (RDH): 60-200GB/s alg-bw depending on concurrent DMA; first
  collective pays ~11.5us trigger latency. Warm-up dummy collectives fail
  the BIR verifier (strided/tiny APs) or delay n1 writes (anti-deps). The
  CC dispatch sem-waits in the in-order gpsimd queue: place AG k at AGOFF[k]
  calls into table k-1's stream.
- Consumer matmuls queue behind ALL layer-A PE work (in-order PE queue), so
  gathers stall on pool-buffer reuse until layer-A drains (~300us):
  GPOOL_BUFS must cover (drain - first gather)/call period.
- Interleaved PSUM accumulation groups in one bank CORRUPT results; keep
  accumulation groups sequential.
- Per-partition-contiguous DRAM stream layout ([B*P, T_A*H]) gives 6KB
  descriptors (~line rate); naive layout caps at ~190GB/s.
- fp8 anywhere in the data path gives ~2% output error (gate is 2e-2) -
  e4m3 rms ~3.6% on N(0,1) data; int16 gather indices cap tables at
  32768 rows (blocks*128*8cores).
- Chip-level thermal throttle: back-to-back runs can measure +15-20%
  (gather calls 8.6 -> 10.3us); cool-down before timing.
"""

import numpy as np
from contextlib import ExitStack

import concourse.bass as bass
import concourse.tile as tile
from concourse import bacc, mybir, library_config
from concourse.bass_utils import run_bass_kernel_spmd
from concourse.masks import make_identity

P = 128
F32 = mybir.dt.float32
F16 = mybir.dt.float16
I16 = mybir.dt.int16
NPF16 = np.float16
EPS = 1e-5

# ---- v8 tuning knobs
TB = (6, 10, 16, 17)      # blocks per layer-A output table (sum must be B=49)
ROWCAP = 1024             # rows per gather call (hard SWDGE carveout cap)
# AG k+1 dispatched after this many gather calls of table k have issued
# (must be late enough that layer A produced table k+1's blocks, early
# enough that AG k+1 completes before table k's gather stream drains).
AGOFF = (1, 5, 1)
MIX_T2 = 11  # table-2 calls deferred into table-3's phase (spreads finish-block PE)
GPOOL_BUFS = 18           # gather pool depth (calls in flight)


def cdiv(a, b):
    return (a + b - 1) // b


# ---------------------------------------------------------------- host prep

def sort_edges_by_dest(e0, e1, ncores, npc):
    """Per core: edge (local_dest, src) arrays sorted by local dest."""
    e0 = np.asarray(e0).astype(np.int64)
    e1 = np.asarray(e1).astype(np.int64)
    out = []
    for c in range(ncores):
        lo = c * npc
        sel = (e0 >= lo) & (e0 < lo + npc)
        ld = e0[sel] - lo
        sr = e1[sel]
        order = np.argsort(ld, kind="stable")
        out.append((ld[order], sr[order]))
    return out


def prep_stream_A(x, per_core, B):
    """Host-gathered layer-A message stream. Per-block tile counts TA[b]
    (max over cores, SPMD-uniform); DRAM padded to TAm per block but only
    TA[b] tiles are transferred/aggregated."""
    blk = []
    TA = [1] * B
    for ld, sr in per_core:
        bid = ld // P
        cnt = np.bincount(bid.astype(np.int64), minlength=B)
        for b in range(B):
            TA[b] = max(TA[b], int(cdiv(max(int(cnt[b]), 1), P)))
        blk.append((ld, sr, bid))
    TAm = max(TA)
    offSA = [0]
    for b in range(B):
        offSA.append(offSA[-1] + TA[b])
    H = x.shape[1]
    out = []
    for ld, sr, bid in blk:
        stream = np.zeros((B, TAm * P, H), NPF16)
        slots = np.full((P, offSA[-1]), 300.0, NPF16)
        for b in range(B):
            m = bid == b
            srcs = sr[m]
            slts = (ld[m] % P).astype(np.float32)
            n = len(srcs)
            stream[b, :n] = x[srcs].astype(NPF16)
            ps = np.full(TA[b] * P, 300.0, np.float32)
            ps[:n] = slts
            slots[:, offSA[b] : offSA[b] + TA[b]] = ps.reshape(TA[b], P).T.astype(NPF16)
        stream = np.ascontiguousarray(
            stream.reshape(B, TAm, P, H).transpose(0, 2, 1, 3)
        ).reshape(B * P, TAm * H)
        out.append(dict(stream=stream, slots=slots))
    return TA, offSA, out


def prep_gather_B(per_core, B, npc, tb_lo, tb_hi):
    """Layer-B edge prep for NT source-range tables.

    Per (dest block b, table k): C[b][k] = cross-core max edge count (shared,
    baked; each core pads its shortfall with idx=0 / slot=300 sentinel rows).
    Per table, the block cells concatenate into one stream that is cut into
    exact ROWCAP-row gather calls (blocks may straddle a call boundary; only
    each table's last call is tile-padded). All rows are valid indices -- HW
    desc-gen charges ~2.4x for -1 skip rows, so dummy row-0 gathers are
    cheaper than skip markers.

    Returns per-table call lists, per-block fragment specs, and per-core
    idx/slot arrays.
    """
    NT = len(tb_lo)
    ncores = len(per_core)
    cells = [[[None] * B for _ in range(NT)] for _ in range(ncores)]
    C = np.zeros((B, NT), np.int64)
    for c, (ld, sr) in enumerate(per_core):
        bid = ld // P
        slot = ld % P
        own_c = sr // npc
        off = sr % npc
        srcblk = off // P
        tbl = np.searchsorted(tb_hi, srcblk, side="right")
        row = own_c * ((np.array(tb_hi) - np.array(tb_lo))[tbl] * P) + (
            off - np.array(tb_lo)[tbl] * P
        )
        for k in range(NT):
            mk = tbl == k
            for b in range(B):
                m = mk & (bid == b)
                cells[c][k][b] = (row[m], slot[m].astype(np.float32))
                C[b, k] = max(C[b, k], int(m.sum()))
    C = np.maximum(C, 1)

    CT = ROWCAP // P * P  # call row capacity, tile-aligned

    # ---- per-table stream layout
    tabs = []  # per table: dict(R[b], S, ncalls, padded[call], ioff[call])
    for k in range(NT):
        R = np.zeros(B + 1, np.int64)
        for b in range(B):
            R[b + 1] = R[b] + C[b, k]
        S = int(R[B])
        ncalls = cdiv(S, CT)
        padded = [CT] * (ncalls - 1) + [cdiv(S - (ncalls - 1) * CT, P) * P]
        tabs.append(dict(k=k, R=R, S=S, ncalls=ncalls, padded=padded))

    # global call list (table-major) + idx prefix offsets
    calls = []  # (k, local_call, padded, ioff)
    ioff = 0
    for t in tabs:
        t["ci0"] = len(calls)
        for lc in range(t["ncalls"]):
            calls.append((t["k"], lc, t["padded"][lc], ioff))
            ioff += t["padded"][lc] // 16
    ncols_idx = ioff

    # per-block fragment/slab specs; completion call per (k, b)
    # stream tile t of table k -> call t // (CT/P), local tile t % (CT/P)
    TPC = CT // P
    blocks = []  # list over (k, b): dict with sall/chunk info
    soff = 0
    for t in tabs:
        k = t["k"]
        R = t["R"]
        for b in range(B):
            tlo = int(R[b]) // P
            thi = cdiv(int(R[b + 1]), P)
            chunks = [(t["ci0"] + tt // TPC, tt % TPC) for tt in range(tlo, thi)]
            blocks.append(
                dict(
                    k=k, b=b, soff=soff, nt=thi - tlo,
                    chunks=chunks, done_ci=chunks[-1][0],
                )
            )
            soff += thi - tlo
    ncols_slot = soff

    out = []
    for c in range(ncores):
        idx = np.zeros((16, ncols_idx), np.int16)
        slots = np.full((P, ncols_slot), 300.0, np.float32)
        for t in tabs:
            k = t["k"]
            R = t["R"]
            Spad = sum(t["padded"])
            vals = np.zeros(Spad, np.int64)
            slv = np.full(Spad, 300.0, np.float32)
            for b in range(B):
                rows, slts = cells[c][k][b]
                n = len(rows)
                vals[int(R[b]) : int(R[b]) + n] = rows
                slv[int(R[b]) : int(R[b]) + n] = slts
            # idx: per call, wrapped-16
            pos = 0
            for lc in range(t["ncalls"]):
                pd = t["padded"][lc]
                io = calls[t["ci0"] + lc][3]
                idx[:, io : io + pd // 16] = (
                    vals[pos : pos + pd].reshape(pd // 16, 16).T.astype(np.int16)
                )
                pos += pd
        for bs in blocks:
            t = tabs[bs["k"]]
            R = t["R"]
            b = bs["b"]
            tlo = int(R[b]) // P
            # slab over stream rows [tlo*P, (tlo+nt)*P), others sentinel
            Spad = sum(t["padded"])
            slv = np.full(Spad + P, 300.0, np.float32)
            rows, slts = cells[c][bs["k"]][b]
            slv[int(R[b]) : int(R[b]) + len(rows)] = slts
            slab = slv[tlo * P : (tlo + bs["nt"]) * P]
            slots[:, bs["soff"] : bs["soff"] + bs["nt"]] = slab.reshape(
                bs["nt"], P
            ).T
        out.append(dict(idx=np.tile(idx, (8, 1)), slots=slots.astype(NPF16)))

    blockspec = tuple(
        (bs["k"], bs["b"], bs["soff"], bs["nt"], tuple(bs["chunks"]), bs["done_ci"])
        for bs in blocks
    )
    return tuple(calls), blockspec, ncols_idx, ncols_slot, out


def prep_all(inputs, ncores=8):
    x = np.asarray(inputs["x"], np.float32)
    N, H = x.shape
    OUT = inputs["Wout"].shape[0]
    npc = N // ncores
    assert npc * ncores == N
    npad = cdiv(npc, P) * P
    B = npad // P
    assert sum(TB) == B, (TB, B)
    tb_lo, tb_hi = [], []
    acc = 0
    for s in TB:
        tb_lo.append(acc)
        acc += s
        tb_hi.append(acc)
        assert s * P * ncores < 32768  # int16 gather index range

    Wl, W0, W1 = (np.asarray(inputs[k], np.float32) for k in ("Wl", "W0", "W1"))
    bl, b0, b1 = (np.asarray(inputs[k], np.float32) for k in ("bl", "b0", "b1"))
    gamma, beta = np.asarray(inputs["gamma"], np.float32), np.asarray(inputs["beta"], np.float32)
    Wout, bout = np.asarray(inputs["Wout"], np.float32), np.asarray(inputs["bout"], np.float32)

    g1, B1 = gamma[1], beta[1]
    g0, B0 = gamma[0], beta[0]
    assert not np.any(B1), "beta of first-applied layer must be 0 (gather fold)"

    WlT_A = Wl[1].T.astype(NPF16)
    W01T_A = (W0[1] + W1[1]).T.astype(NPF16)
    bias_A = bl[1] + b0[1] + b1[1]
    WlT_B = (g1[:, None] * Wl[0].T).astype(NPF16)
    W0T_B = (g1[:, None] * W0[0].T).astype(NPF16)
    W1T_B = W1[0].T.astype(NPF16)
    bias_B = bl[0] + b0[0] + b1[0] + B1 @ W0[0].T
    WoutT = (g0[:, None] * Wout.T).astype(NPF16)
    bout_e = bout + B0 @ Wout.T

    e2 = np.asarray(inputs["edge_r2"])
    e1e = np.asarray(inputs["edge_r1"])
    pcA = sort_edges_by_dest(e2[0], e2[1], ncores, npc)
    pcB = sort_edges_by_dest(e1e[0], e1e[1], ncores, npc)
    TA, offSA, packA = prep_stream_A(x, pcA, B)
    callspec, blockspec, ncols_idx, ncols_slot, packB = prep_gather_B(
        pcB, B, npc, tb_lo, tb_hi
    )

    TMAX = max(max(TA), max(bs[3] for bs in blockspec))
    iota = np.tile(np.arange(P, dtype=np.float32), (P, TMAX)).astype(NPF16)

    cfg = dict(
        N=N, H=H, OUT=OUT, npc=npc, npad=npad, B=B,
        tb_lo=tuple(tb_lo), tb_hi=tuple(tb_hi),
        T_A=tuple(TA), offSA=tuple(offSA),
        callspec=callspec, blockspec=blockspec,
        ncols_idx=ncols_idx, ncols_slot=ncols_slot,
        TMAX=TMAX, ncores=ncores,
        has_bias_A=bool(np.any(bias_A)), has_bias_B=bool(np.any(bias_B)),
        has_bout=bool(np.any(bout_e)),
    )

    in_maps = []
    for c in range(ncores):
        xT_own = np.zeros((H, npad), np.float32)
        xT_own[:, :npc] = x[c * npc : (c + 1) * npc].T
        m = dict(
            gA_stream=packA[c]["stream"], slotA=packA[c]["slots"],
            xT_own=xT_own.astype(NPF16),
            idxB=packB[c]["idx"], slotB=packB[c]["slots"],
            iota=iota,
            WlT_A=WlT_A, W01T_A=W01T_A,
            WlT_B=WlT_B, W0T_B=W0T_B, W1T_B=W1T_B, WoutT=WoutT,
            bias_A=bias_A.reshape(1, H), bias_B=bias_B.reshape(1, H),
            bout_e=bout_e.reshape(1, OUT),
        )
        in_maps.append(m)
    return cfg, in_maps


# ---------------------------------------------------------------- device build

def build_nc(cfg):
    H, OUT, npad, B = cfg["H"], cfg["OUT"], cfg["npad"], cfg["B"]
    TA = cfg["T_A"]
    offSA = cfg["offSA"]
    TAm = max(TA)
    tb_lo, tb_hi = cfg["tb_lo"], cfg["tb_hi"]
    NT = len(tb_lo)
    callspec = cfg["callspec"]
    ncores = cfg["ncores"]
    KH = H // P
    CALL_TILES = max(cs[2] for cs in callspec) // P  # padded tiles per call

    nc = bacc.Bacc(
        "TRN2", target_bir_lowering=False, debug=False, num_devices=ncores,
    )

    def din(name, shape, dt=F16):
        return nc.dram_tensor(name, shape, dt, kind="ExternalInput")

    gA_stream = din("gA_stream", [B * P, TAm * H])
    slotA = din("slotA", [P, offSA[-1]])
    xT_own = din("xT_own", [H, npad])
    idxB = din("idxB", [P, cfg["ncols_idx"]], I16)
    slotB = din("slotB", [P, cfg["ncols_slot"]])
    iota = din("iota", [P, cfg["TMAX"] * P])
    WlT_A = din("WlT_A", [H, H])
    W01T_A = din("W01T_A", [H, H])
    WlT_B = din("WlT_B", [H, H])
    W0T_B = din("W0T_B", [H, H])
    W1T_B = din("W1T_B", [H, H])
    WoutT = din("WoutT", [H, OUT])
    bias_A = din("bias_A", [1, H], F32)
    bias_B = din("bias_B", [1, H], F32)
    bout_e = din("bout_e", [1, OUT], F32)

    blockspec = cfg["blockspec"]
    hk = [(tb_hi[k] - tb_lo[k]) * P for k in range(NT)]
    n1t = [nc.dram_tensor(f"n1_{k}", [hk[k], H], F16) for k in range(NT)]
    tbl = [
        nc.dram_tensor(f"tbl_{k}", [ncores * hk[k], H], F16, addr_space="Shared")
        for k in range(NT)
    ]
    out_own = nc.dram_tensor("out_own", [npad, OUT], F32, kind="ExternalOutput")

    with tile.TileContext(nc) as tc:
        nc.gpsimd.load_library(library_config.mlp)
        with ExitStack() as ctx:
            const = ctx.enter_context(tc.tile_pool(name="const", bufs=1))
            idxp = ctx.enter_context(tc.tile_pool(name="idxp", bufs=1))
            gpoolA = ctx.enter_context(tc.tile_pool(name="gpoolA", bufs=4))
            gpoolB = ctx.enter_context(tc.tile_pool(name="gpoolB", bufs=GPOOL_BUFS))
            sall = ctx.enter_context(tc.tile_pool(name="sall", bufs=2))
            work = ctx.enter_context(tc.tile_pool(name="work", bufs=2))
            ntp = ctx.enter_context(tc.tile_pool(name="ntp", bufs=2))
            stat = ctx.enter_context(tc.tile_pool(name="stat", bufs=3))
            aps = ctx.enter_context(tc.tile_pool(name="aps", bufs=2, space="PSUM"))
            zps = ctx.enter_context(tc.tile_pool(name="zps", bufs=2, space="PSUM"))
            tps = ctx.enter_context(tc.tile_pool(name="tps", bufs=2, space="PSUM"))
            ops = ctx.enter_context(tc.tile_pool(name="ops", bufs=2, space="PSUM"))

            # ---- gather pool buffers, memset once (gpsimd is idle pre-AG0;
            # unwritten tail rows would otherwise be NaN-capable garbage)
            gtiles = []
            for i in range(GPOOL_BUFS):
                g = gpoolB.tile([P, CALL_TILES, H], F16, tag="gB")
                nc.gpsimd.memset(g[:], 0.0)
                gtiles.append(g)

            # ---- constants / persistent tables (order matters: block 0's
            # aggregation needs iota+slotA+stream first; weights only at z)
            iota_t = const.tile([P, cfg["TMAX"] * P], F16)
            nc.sync.dma_start(iota_t[:], iota[:])
            ident = const.tile([P, P], F16)
            make_identity(nc, ident[:])
            eps_col = const.tile([P, 1], F32)
            nc.vector.memset(eps_col[:], EPS)

            slotA_t = idxp.tile(list(slotA.shape), F16, tag="slotA_sb")
            nc.sync.dma_start(slotA_t[:], slotA[:])

            def load_w(t, KN):
                w = const.tile([P, KH, KN], F16, tag=t.name + "_sb")
                nc.sync.dma_start(w[:], t[:].rearrange("(k p) n -> p k n", p=P))
                return w

            wlA = load_w(WlT_A, H)
            w01A = load_w(W01T_A, H)
            wlB = load_w(WlT_B, H)
            w0B = load_w(W0T_B, H)
            w1B = load_w(W1T_B, H)
            wout = load_w(WoutT, OUT)
            if cfg["has_bias_A"]:
                biasA_t = const.tile([1, H], F32)
                nc.sync.dma_start(biasA_t[:], bias_A[:])
            else:
                biasA_t = None
            if cfg["has_bias_B"]:
                biasB_t = const.tile([1, H], F32)
                nc.sync.dma_start(biasB_t[:], bias_B[:])
            else:
                biasB_t = None
            if cfg["has_bout"]:
                bout_t = const.tile([1, OUT], F32)
                nc.sync.dma_start(bout_t[:], bout_e[:])
            else:
                bout_t = None

            def load_flat(t, dt, eng):
                s = idxp.tile(list(t.shape), dt, tag=t.name + "_sb")
                eng.dma_start(s[:], t[:])
                return s

            # layer-B index/slot tables aren't needed until the first gather
            # consumption (~140us in) -- keep them off the startup sync queue
            idxB_t = load_flat(idxB, I16, nc.scalar)
            slotB_t = load_flat(slotB, F16, nc.scalar)

            # Persistent xT table [feat(p) x (B, KH) x dest]. Chunk 0 on the
            # sync queue (block 0's z needs it); the rest via scalar HWDGE so
            # startup DMA doesn't delay the first stream loads.
            xT_tab = const.tile([P, B, KH, P], F16)
            XCH = cdiv(B, 7)
            for ci in range(XCH):
                b0 = ci * 7
                b1 = min(B, (ci + 1) * 7)
                eng = nc.sync if ci == 0 else nc.scalar
                for k in range(KH):
                    eng.dma_start(
                        xT_tab[:, b0:b1, k, :],
                        xT_own[k * P : (k + 1) * P, b0 * P : b1 * P].rearrange(
                            "p (b d) -> p b d", d=P
                        ),
                    )
            # Persistent transposed layer-A output [feat(p) x (B,KH) x dest].
            n1T_tab = const.tile([P, B, KH, P], F16)
            # Layer-B per-block aggregate accumulator (f16, across table phases)
            aggSB = const.tile([P, B, KH, P], F16)

            # ---------------- shared per-block pieces ----------------

            def build_sall(slot_t, base, nt, tag):
                s = sall.tile([P, cfg["TMAX"], P], F16, tag=tag)
                nc.vector.tensor_tensor(
                    out=s[:, 0:nt, :],
                    in0=slot_t[:, base : base + nt].to_broadcast([P, nt, P])[:],
                    in1=iota_t[:, 0 : nt * P].rearrange("p (t d) -> p t d", t=nt),
                    op=mybir.AluOpType.is_equal,
                )
                return s

            def edge_mms(chunks, s_t):
                """aggT accumulation: agg[:, h, :] += G_half.T @ S per edge tile.
                Sequential accumulation groups only (interleaved groups
                corrupt PSUM)."""
                agg = aps.tile([P, KH, P], F32, tag="agg", space="PSUM")
                nt = len(chunks)
                for h in range(KH):
                    for i, (gt, ch) in enumerate(chunks):
                        nc.tensor.matmul(
                            agg[:, h, :],
                            lhsT=gt[:, ch, h * P : (h + 1) * P],
                            rhs=s_t[:, i, :],
                            start=(i == 0), stop=(i == nt - 1),
                        )
                return agg

            def z_part(agg_lhs, terms, wl, tag):
                """z matmuls from f16 lhsT tiles. Returns z PSUM tile."""
                z = zps.tile([P, H], F32, tag="z", space="PSUM")
                mats = [(agg_lhs, None, wl)] + terms
                mm = [(t, b_, w, k) for (t, b_, w) in mats for k in range(KH)]
                for i, (t, b_, w, k) in enumerate(mm):
                    lhs = t[:, k, :] if b_ is None else t[:, b_, k, :]
                    nc.tensor.matmul(
                        z[:], lhsT=lhs, rhs=w[:, k, :],
                        start=(i == 0), stop=(i == len(mm) - 1),
                    )
                return z

            def ln_part(z, bias_t):
                """relu + LN stats + normalized n_t [P,H] f16."""
                zr = work.tile([P, H], F32, tag="zr")
                s1 = stat.tile([P, 1], F32, tag="s1")
                if bias_t is not None:
                    zb = work.tile([P, H], F32, tag="zb")
                    nc.vector.tensor_tensor(
                        out=zb[:], in0=z[:], in1=bias_t[:].to_broadcast([P, H])[:],
                        op=mybir.AluOpType.add,
                    )
                    zsrc = zb
                else:
                    zsrc = z
                nc.scalar.activation(
                    zr[:], zsrc[:], mybir.ActivationFunctionType.Relu, accum_out=s1[:],
                )
                sq = work.tile([P, H], F32, tag="sq")
                s2 = stat.tile([P, 1], F32, tag="s2")
                nc.scalar.activation(
                    sq[:], zr[:], mybir.ActivationFunctionType.Square, accum_out=s2[:],
                )
                mu = stat.tile([P, 1], F32, tag="mu")
                nc.vector.tensor_scalar_mul(mu[:], s1[:], 1.0 / H)
                ex2 = stat.tile([P, 1], F32, tag="ex2")
                nc.vector.tensor_scalar_mul(ex2[:], s2[:], 1.0 / H)
                mu2 = stat.tile([P, 1], F32, tag="mu2")
                nc.vector.tensor_tensor(out=mu2[:], in0=mu[:], in1=mu[:], op=mybir.AluOpType.mult)
                var = stat.tile([P, 1], F32, tag="var")
                nc.vector.tensor_tensor(out=var[:], in0=ex2[:], in1=mu2[:], op=mybir.AluOpType.subtract)
                std = stat.tile([P, 1], F32, tag="std")
                nc.scalar.activation(
                    std[:], var[:], mybir.ActivationFunctionType.Sqrt, bias=eps_col[:, 0:1],
                )
                rstd = stat.tile([P, 1], F32, tag="rstd")
                nc.vector.reciprocal(rstd[:], std[:])
                nmr = stat.tile([P, 1], F32, tag="nmr")
                nc.vector.scalar_tensor_tensor(
                    out=nmr[:], in0=mu[:], scalar=-1.0, in1=rstd[:],
                    op0=mybir.AluOpType.mult, op1=mybir.AluOpType.mult,
                )
                n_t = ntp.tile([P, H], F16, tag="n_t")
                nc.vector.tensor_scalar(
                    out=n_t[:], in0=zr[:], scalar1=rstd[:, 0:1], scalar2=nmr[:, 0:1],
                    op0=mybir.AluOpType.mult, op1=mybir.AluOpType.add,
                )
                return n_t

            def transpose_pair(n_t, tag="tp"):
                """PE-transpose n_t [dest, H] into [feat(p), KH, dest] PSUM pair."""
                tp = tps.tile([P, KH, P], F16, tag="tp", space="PSUM")
                for k in range(KH):
                    nc.tensor.transpose(tp[:, k, :], n_t[:, k * P : (k + 1) * P], ident[:])
                return tp

            # ---------------- layer A (pipelined) ----------------

            def stream_load(b):
                g = gpoolA.tile([P, TAm, H], F16, tag="gA")
                nc.sync.dma_start(
                    g[:, 0 : TA[b], :].rearrange("p t f -> p (t f)"),
                    gA_stream[b * P : (b + 1) * P, 0 : TA[b] * H],
                )
                return g

            def n1_write(b, n_t):
                k = 0
                while b >= tb_hi[k]:
                    k += 1
                bb = b - tb_lo[k]
                nc.sync.dma_start(n1t[k][bb * P : (bb + 1) * P, :], n_t[:])

            gA = {}
            for b in range(min(3, B)):
                gA[b] = stream_load(b)
            sA = {0: build_sall(slotA_t, 0, TA[0], "sA")}
            aggs = {0: edge_mms([(gA[0], t) for t in range(TA[0])], sA[0])}
            nts = {}
            for b in range(B):
                if b + 1 < B:
                    if b + 3 < B:
                        gA[b + 3] = stream_load(b + 3)
                    sA[b + 1] = build_sall(
                        slotA_t, offSA[b + 1], TA[b + 1], "sA")
                    aggs[b + 1] = edge_mms(
                        [(gA[b + 1], t) for t in range(TA[b + 1])], sA[b + 1]
                    )
                    gA.pop(b, None); sA.pop(b, None)
                agg = aggs.pop(b)
                aT = work.tile([P, KH, P], F16, tag="aT")
                nc.scalar.activation(aT[:], agg[:], mybir.ActivationFunctionType.Copy)
                z = z_part(aT, [(xT_tab, b, w01A)], wlA, "A")
                if b >= 1:
                    tp = transpose_pair(nts[b - 1], "tpA")
                    nc.scalar.activation(
                        n1T_tab[:, b - 1, :, :], tp[:],
                        mybir.ActivationFunctionType.Copy)
                    n1_write(b - 1, nts.pop(b - 1))
                nts[b] = ln_part(z, biasA_t)
            tp = transpose_pair(nts[B - 1], "tpA")
            nc.scalar.activation(
                n1T_tab[:, B - 1, :, :], tp[:],
                mybir.ActivationFunctionType.Copy)
            n1_write(B - 1, nts.pop(B - 1))

            # ---------------- gather + AG schedule on the gpsimd queue ------
            def do_ag(k):
                nc.gpsimd.collective_compute(
                    "AllGather", mybir.AluOpType.bypass,
                    replica_groups=[list(range(ncores))],
                    ins=[n1t[k][:].opt()], outs=[tbl[k][:].opt()],
                )

            def do_gather(ci, g):
                k, lc, padded, ioff = callspec[ci]
                nc.gpsimd.dma_gather(
                    g[:, 0 : padded // P, :], tbl[k][:],
                    idxB_t[:, ioff : ioff + padded // 16],
                    padded, padded, H,
                )

            # consumer: per-block aggregation over (possibly two) call tiles,
            # accumulate into aggSB; table NT-1 triggers z/LN/out.
            call_tile = {}

            def consume_block(bs):
                k, b, soff, nt, chunks, done_ci = bs
                s_t = build_sall(slotB_t, soff, nt, "sB")
                agg = edge_mms(
                    [(call_tile[cci], t) for cci, t in chunks], s_t
                )
                if k == 0:
                    nc.vector.tensor_copy(aggSB[:, b, :, :], agg[:])
                else:
                    nc.vector.tensor_tensor(
                        out=aggSB[:, b, :, :], in0=aggSB[:, b, :, :],
                        in1=agg[:], op=mybir.AluOpType.add,
                    )
                if k == NT - 1:
                    finish_block(b)

            def finish_block(b):
                aT = work.tile([P, KH, P], F16, tag="aTB")
                nc.scalar.activation(
                    aT[:], aggSB[:, b, :, :], mybir.ActivationFunctionType.Copy)
                z = z_part(aT, [(n1T_tab, b, w0B), (xT_tab, b, w1B)], wlB, "B")
                n_t = ln_part(z, biasB_t)
                tp = transpose_pair(n_t, "tpB")
                n2T = work.tile([P, KH, P], F16, tag="n2T")
                nc.scalar.activation(n2T[:], tp[:], mybir.ActivationFunctionType.Copy)
                op = ops.tile([P, OUT], F32, tag="op", space="PSUM")
                for k in range(KH):
                    nc.tensor.matmul(
                        op[:], lhsT=n2T[:, k, :], rhs=wout[:, k, :],
                        start=(k == 0), stop=(k == KH - 1),
                    )
                ot = work.tile([P, OUT], F32, tag="ot")
                if bout_t is not None:
                    nc.vector.tensor_tensor(
                        out=ot[:], in0=op[:], in1=bout_t[:].to_broadcast([P, OUT])[:],
                        op=mybir.AluOpType.add,
                    )
                else:
                    nc.scalar.activation(
                        ot[:], op[:], mybir.ActivationFunctionType.Copy)
                nc.sync.dma_start(out_own[b * P : (b + 1) * P, :], ot[:])

            # emit schedule: AG0 first, then gather calls; AG k+1 dispatched
            # AGOFF[k] calls into table k's stream. The last MIX_T2 calls of
            # table NT-2 are deferred and interleaved 1:2 with table NT-1's
            # calls so per-block completions (z/LN/out, ~3.5us PE each) spread
            # over the whole post-AG3 window instead of bunching.
            starts = {}
            for ci, cs in enumerate(callspec):
                starts.setdefault(cs[0], ci)
            agpos = {k + 1: starts[k] + AGOFF[k] for k in range(NT - 1)}
            t2_calls = [ci for ci, cs in enumerate(callspec) if cs[0] == NT - 2]
            t3_calls = [ci for ci, cs in enumerate(callspec) if cs[0] == NT - 1]
            defer = t2_calls[len(t2_calls) - MIX_T2 :] if MIX_T2 else []
            dset = set(defer) | set(t3_calls)
            prefix = [ci for ci in range(len(callspec)) if ci not in dset]
            inter = []
            di = ti = 0
            nd, nt3 = len(defer), len(t3_calls)
            while di < nd or ti < nt3:
                # proportional weave: keep di/nd ~ ti/nt3, t2 slightly ahead
                if di < nd and (ti >= nt3 or di * nt3 <= ti * nd):
                    inter.append(defer[di]); di += 1
                elif ti < nt3:
                    inter.append(t3_calls[ti]); ti += 1
            order = prefix + inter
            emit_pos = {ci: e for e, ci in enumerate(order)}
            consume_at = {}
            for bs in blockspec:
                last = max(emit_pos[cci] for cci, _ in bs[4])
                consume_at.setdefault(last, []).append(bs)
            do_ag(0)
            for e, ci in enumerate(order):
                for k, pos in agpos.items():
                    if pos == ci:
                        do_ag(k)
                call_tile[ci] = gtiles[e % GPOOL_BUFS]
                do_gather(ci, call_tile[ci])
                for bs in consume_at.get(e, ()):
                    consume_block(bs)

    nc.compile()
    return nc


def run(inputs, ncores=8, nc_cache={}, trace=False, tmpdir=None):
    cfg, in_maps = prep_all(inputs, ncores)
    key = tuple(sorted((k, str(v)) for k, v in cfg.items()))
    if key not in nc_cache:
        nc_cache[key] = build_nc(cfg)
    nc = nc_cache[key]
    res = run_bass_kernel_spmd(
        nc, in_maps, core_ids=list(range(ncores)), trace=trace, tmpdir=tmpdir
    )
    npc = cfg["npc"]
    out = np.concatenate(
        [res.results[c]["out_own"][:npc] for c in range(ncores)], axis=0
    )
    return (out, res) if trace else out


def kernel(**inputs):
    """Full-input entry point: shards across the 8 NeuronCores internally and
    returns the full [N, OUT] float32 output."""
    return np.ascontiguousarray(run(inputs, 8).astype(np.float32))
